# revision 1
# baseline (speedup 1.0000x reference)
"""MoE SwiGLU experts kernel for Trainium2 (8 NeuronCores, expert-parallel).

Each core owns one expert e. Host does the dispatch (gathers tokens whose
top-k includes e, dedups with summed combine weights), transposes operands
so every matmul contraction dim lands on SBUF partitions, and pads the
token batch to capacity C. Device computes the SwiGLU MLP for its expert:

    gateT = gate_w[e] @ x_eT          # [I, C]  (contract H)
    upT   = up_w[e]   @ x_eT          # [I, C]
    interT = silu(gateT) * upT        # [I, C]
    outT  = down_w[e]  @ interT       # [H, C]  (contract I)

Host scatter-adds w * outT.T rows into the [T, H] output.
Matmuls run in bf16 with fp32 PSUM accumulation.

Hardware sync-wait slots per instruction are scarce (walrus rejects
kernels that need too many), so the structure keeps every instruction's
dependency fan-in tiny:
- all three weight tensors and the token slab are fully SBUF-resident and
  written once, so their DMAs carry no WAR/WAW waits;
- DVE tensor ops depend on exactly one semaphore (both multiplicands are
  produced by ACT; the destination is written once);
- slot-rotating buffers are only ever rewritten by ACT, which has enough
  wait slots for {producer, WAR, own} sets;
- output stores are batched into 8 DMAs so each lands on a fresh HW lane.
"""

import numpy as np


def _build_bass(C: int, H: int, I: int):
    from contextlib import ExitStack

    import concourse.bass as bass
    import concourse.mybir as mybir
    import concourse.tile as tile

    f32 = mybir.dt.float32
    bf16 = mybir.dt.bfloat16
    P = 128
    KH = H // P  # 16
    KI = I // P  # 11

    chunks = []
    off = 0
    while off < C:
        w = min(512, C - off)
        chunks.append((off, w))
        off += w

    nc = bass.Bass(dynamic_dma_scratch_size=8192)
    xT_d = nc.dram_tensor("xT", [H, C], bf16, kind="ExternalInput")
    gwT_d = nc.dram_tensor("gwT", [H, I], bf16, kind="ExternalInput")
    uwT_d = nc.dram_tensor("uwT", [H, I], bf16, kind="ExternalInput")
    dwT_d = nc.dram_tensor("dwT", [I, H], bf16, kind="ExternalInput")
    outT_d = nc.dram_tensor("outT", [H, C], bf16, kind="ExternalOutput")

    x3 = xT_d[:].rearrange("(o p) c -> p o c", p=P)
    gw3 = gwT_d[:].rearrange("(o p) i -> p o i", p=P)
    uw3 = uwT_d[:].rearrange("(o p) i -> p o i", p=P)
    dw3 = dwT_d[:].rearrange("(o p) h -> p o h", p=P)
    o3 = outT_d[:].rearrange("(o p) c -> p o c", p=P)

    with ExitStack() as ctx:
        tc = ctx.enter_context(tile.TileContext(nc))
        wpool = ctx.enter_context(tc.tile_pool(name="w", bufs=1))
        xpool = ctx.enter_context(tc.tile_pool(name="x", bufs=1))
        ipool = ctx.enter_context(tc.tile_pool(name="inter", bufs=1))
        opool = ctx.enter_context(tc.tile_pool(name="out", bufs=1))
        ppool = ctx.enter_context(tc.tile_pool(name="psum", bufs=1, space="PSUM"))
        ppool2 = ctx.enter_context(tc.tile_pool(name="psum2", bufs=2, space="PSUM"))

        # weights resident: every load writes a fresh region
        gw_sb = wpool.tile([P, KH, I], bf16, name="gw_sb")
        uw_sb = wpool.tile([P, KH, I], bf16, name="uw_sb")
        dw_sb = wpool.tile([P, KI, H], bf16, name="dw_sb")
        sw_dmas = []
        for kh in range(KH):
            nc.gpsimd.dma_start(gw_sb[:, kh], gw3[:, kh])
            nc.gpsimd.dma_start(uw_sb[:, kh], uw3[:, kh])
        for ki in range(KI):
            sw_dmas.append(nc.gpsimd.dma_start(dw_sb[:, ki], dw3[:, ki]))
        # x and out share one hand-allocated slab (x is dead after phase 1;
        # both hazard directions resolve to the PE semaphore)
        xo_arena = nc.alloc_sbuf_tensor("xo_arena", [P, KH * C], bf16)
        xo_off = nc.lookup_mloc(xo_arena).addr
        x_sb = nc.alloc_sbuf_tensor_at("x_sb", [P, KH, C], bf16, offset=xo_off)[:]
        out_sb = nc.alloc_sbuf_tensor_at(
            "out_sb", [P, KH, C], bf16, offset=xo_off
        )[:]
        x_dma = nc.sync.dma_start(x_sb[:], x3)
        inter_sb = ipool.tile([P, KI, C], bf16, name="inter_sb")
        puc_sb = ipool.tile([P, KI, C], bf16, name="puc_sb")
        fence_t = ipool.tile([P, 16], bf16, name="fence_t")
        last_tt = [None]

        # ---- phase 1: interT = silu(gateT) * upT ----
        for im in range(KI):
            pg = [
                ppool.tile([P, w], f32, tag=f"a{j}", name=f"pg{j}")
                for j, (_, w) in enumerate(chunks)
            ]
            pu = [
                ppool.tile([P, w], f32, tag=f"b{j}", name=f"pu{j}")
                for j, (_, w) in enumerate(chunks)
            ]
            for kh in range(KH):
                for j, (o, w) in enumerate(chunks):
                    nc.tensor.matmul(
                        pg[j][:],
                        gw_sb[:, kh, im * P : (im + 1) * P],
                        x_sb[:, kh, o : o + w],
                        start=(kh == 0),
                        stop=(kh == KH - 1),
                    )
            for kh in range(KH):
                for j, (o, w) in enumerate(chunks):
                    nc.tensor.matmul(
                        pu[j][:],
                        uw_sb[:, kh, im * P : (im + 1) * P],
                        x_sb[:, kh, o : o + w],
                        start=(kh == 0),
                        stop=(kh == KH - 1),
                    )
            for j, (o, w) in enumerate(chunks):
                # ACT evacuates both PSUM tiles into write-once resident
                # SBUF buffers ({PE} is each copy's only wait); the DVE
                # multiply then reads two ACT products - one semaphore
                nc.scalar.activation(
                    inter_sb[:, im, o : o + w],
                    pg[j][:],
                    mybir.ActivationFunctionType.Silu,
                )
                nc.scalar.copy(puc_sb[:, im, o : o + w], pu[j][:])
                last_tt[0] = nc.vector.tensor_mul(
                    inter_sb[:, im, o : o + w],
                    inter_sb[:, im, o : o + w],
                    puc_sb[:, im, o : o + w],
                )

        # ---- phase 2: outT = down_w @ interT ----
        # absorb the x-DMA completion into the ACT proc via a pure sync
        # edge, so the out copies' WAW-vs-x fence is already observed and
        # each copy's only wait is the PE semaphore
        from concourse.tile import add_dep_helper

        fence = nc.scalar.copy(fence_t[:1, 0:8], fence_t[:1, 0:8])
        add_dep_helper(fence.ins, x_dma.ins, sync=True, reason="x lane fence")
        # sacrificial first reader of the aliased slab: takes the one-time
        # WAW-vs-x fence so the real output stores don't carry it
        scr_d = nc.dram_tensor("scr", [1, 16], bf16)
        dummy_store = nc.sync.dma_start(scr_d[:], x_sb[0:1, 0, 0:16])
        fence2 = nc.scalar.copy(fence_t[:1, 8:16], fence_t[:1, 8:16])
        add_dep_helper(fence2.ins, dummy_store.ins, sync=True, reason="slab fence")
        # absorb the last TT's DVE tick into the PE proc so phase-2 matmuls
        # wait only on their dw DMA lane
        pe_fence = nc.tensor.ldweights(gw_sb[:, 0, 0:1])
        add_dep_helper(pe_fence.ins, last_tt[0].ins, sync=True, reason="pe fence")
        hw_dmas = [x_dma, dummy_store]
        tail_insts = []
        last_mm = []
        for hm in range(KH):
            po = [
                ppool2.tile([P, w], f32, tag=f"c{j}", name=f"po{j}")
                for j, (_, w) in enumerate(chunks)
            ]
            for ki in range(KI):
                for j, (o, w) in enumerate(chunks):
                    last_mm.append(nc.tensor.matmul(
                        po[j][:],
                        dw_sb[:, ki, hm * P : (hm + 1) * P],
                        inter_sb[:, ki, o : o + w],
                        start=(ki == 0),
                        stop=(ki == KI - 1),
                    ))
                    del last_mm[:-1]
            for j, (o, w) in enumerate(chunks):
                tail_insts.append(
                    nc.scalar.copy(out_sb[:, hm, o : o + w], po[j][:])
                )
            # 7 stores (6x2 rows + final 4): with the dw load that is 8 HW
            # DMAs total - each lands on a fresh HW lane (no ring wait)
            if hm % 2 == 1 and hm < 10:
                hw_dmas.append(nc.sync.dma_start(
                    o3[:, hm - 1 : hm + 1, :], out_sb[:, hm - 1 : hm + 1]
                ))
            elif hm == 15:
                hw_dmas.append(nc.sync.dma_start(
                    o3[:, 10:16, :], out_sb[:, 10:16]
                ))

        # pre-drain: absorb every proc's final tick into the SP sequencer
        # one sync edge at a time, so the kernel-tail drain's waits (which
        # would exceed the instruction's wait slots) are all elided
        for insts in (sw_dmas, hw_dmas, [last_tt[0]], tail_insts[-2:], last_mm[-1:]):
            for bi in insts:
                if bi is None:
                    continue
                nop = nc.sync.nop()
                add_dep_helper(nop.ins, bi.ins, sync=True, reason="pre-drain")

    return nc


def kernel(hidden_states, top_k_index, top_k_weights, gate_w, up_w, down_w):
    import ml_dtypes
    from concourse.bass_utils import run_bass_kernel_spmd

    bf = ml_dtypes.bfloat16
    hs = np.ascontiguousarray(np.asarray(hidden_states, dtype=np.float32))
    tki = np.asarray(top_k_index)
    tkw = np.asarray(top_k_weights, dtype=np.float32)
    gw = np.asarray(gate_w, dtype=np.float32)
    uw = np.asarray(up_w, dtype=np.float32)
    dw = np.asarray(down_w, dtype=np.float32)

    T, H = hs.shape
    E, I, _ = gw.shape

    tok_lists, w_lists = [], []
    for e in range(E):
        mask = tki == e
        toks = np.nonzero(mask.any(axis=1))[0]
        w = (tkw * mask).sum(axis=1)[toks].astype(np.float32)
        tok_lists.append(toks)
        w_lists.append(w)

    # fixed capacity (PSUM/SBUF plan is sized for C=1024); experts with
    # more tokens (never happens for balanced routing at T=4096, K=2, E=8)
    # spill into additional SPMD rounds
    C = 1024
    n_rounds = max(1, -(-max(len(t) for t in tok_lists) // C))

    wT = [np.ascontiguousarray(gw[e].T.astype(bf)) for e in range(E)]
    uT = [np.ascontiguousarray(uw[e].T.astype(bf)) for e in range(E)]
    dT = [np.ascontiguousarray(dw[e].T.astype(bf)) for e in range(E)]

    nc = _build_bass(C, H, I)
    out = np.zeros((T, H), np.float32)
    global _last_results
    for r in range(n_rounds):
        in_maps = []
        for e in range(E):
            toks = tok_lists[e][r * C : (r + 1) * C]
            xT = np.zeros((H, C), bf)
            xT[:, : len(toks)] = hs[toks].T.astype(bf)
            in_maps.append(
                {"xT": xT, "gwT": wT[e], "uwT": uT[e], "dwT": dT[e]}
            )
        res = run_bass_kernel_spmd(nc, in_maps, core_ids=list(range(E)))
        _last_results = res
        for e in range(E):
            toks = tok_lists[e][r * C : (r + 1) * C]
            n = len(toks)
            if n == 0:
                continue
            outT_e = np.asarray(res.results[e]["outT"]).astype(np.float32)
            out[toks] += w_lists[e][r * C : r * C + n, None] * outT_e[:, :n].T
    return out



# revision 42
# speedup vs baseline: 1.3771x; 1.3771x over previous
"""MoE SwiGLU experts kernel for Trainium2 (8 NeuronCores, expert-parallel).

Each core owns one expert e. Host does the dispatch (gathers tokens whose
top-k includes e, dedups with summed combine weights), splits every matmul
operand into two fp8-e4m3 planes (hi = fp8(v), lo = fp8(v - hi), with
power-of-two scales so the uniform-[-1/sqrt(H)] weights clear the fp8
denormal floor), and pre-swizzles everything to partition-major layouts.
Device computes the SwiGLU MLP with fp8 DoubleRow matmuls: each
instruction contracts two 128-row k-subtiles, and the three significant
cross-products (hi*hi, hi*lo, lo*hi) recover ~bf16 accuracy at half the
PE cycles of bf16:

    gateT = gate_w[e] @ x_eT          # [I, C]  (contract H, 24 instrs/chain)
    upT   = up_w[e]   @ x_eT          # [I, C]
    interT = silu(gateT) * upT        # [I, C]  requantized to fp8 hi/lo
    outT  = down_w[e]  @ interT       # [H, C]  (contract I, 17 instrs/chain)

The intermediate is stored hi/lo-interleaved [P, KI, 2, C] so phase-2
moving operands pair adjacent k-subtiles (planes strided) and the odd
11th subtile pairs its own hi/lo planes against a duplicated stationary
slot. Host scatter-adds w * outT.T rows into the [T, H] output.

Hardware sync-wait slots per instruction are scarce (walrus rejects
kernels that need too many), so the structure keeps every instruction's
dependency fan-in tiny:
- all weight planes and the token slab are fully SBUF-resident and
  written once, so their DMAs carry no WAR/WAW waits;
- the evac pipeline alternates ACT/DVE so same-engine program order
  subsumes most deps; slot-rotating staging is rewritten only after its
  single cross-engine reader ran;
- output stores are batched so each lands on a fresh HW DMA lane.
"""

import numpy as np

# power-of-two scales: keep fp8 operands out of the denormal floor and
# under the e4m3 max (240); all folded back via ACT scale params
S_W = float(2.0**12)
S_X = float(2.0**4)
S_I = float(2.0**3)

# CoreSim-only: initialize the fence scratch so the interpreter's
# uninitialized-read check passes. The device build must NOT set this -
# fence reads of unwritten scratch are intentionally dependency-free.
_INIT_FENCE = False


def _build_bass(C: int, H: int, I: int):
    from contextlib import ExitStack

    import concourse.bass as bass
    import concourse.mybir as mybir
    import concourse.tile as tile
    from concourse.tile import add_dep_helper

    f32 = mybir.dt.float32
    bf16 = mybir.dt.bfloat16
    f8 = mybir.dt.float8e4
    P = 128
    KH = H // P  # 16
    KI = I // P  # 11
    KI2 = KI + 1  # down stationary gets a duplicate of the odd last subtile
    DR = mybir.MatmulPerfMode.DoubleRow

    chunks = []
    off = 0
    while off < C:
        w = min(512, C - off)
        chunks.append((off, w))
        off += w
    NJ = len(chunks)

    nc = bass.Bass(dynamic_dma_scratch_size=8192)
    # all inputs pre-swizzled on host to [P, ...] partition-major layouts
    xhi_d = nc.dram_tensor("xhi", [P, KH * C], f8, kind="ExternalInput")
    xlo_d = nc.dram_tensor("xlo", [P, KH * C], f8, kind="ExternalInput")
    ghi_d = nc.dram_tensor("ghi", [P, KI * KH * P], f8, kind="ExternalInput")
    glo_d = nc.dram_tensor("glo", [P, KI * KH * P], f8, kind="ExternalInput")
    uhi_d = nc.dram_tensor("uhi", [P, KI * KH * P], f8, kind="ExternalInput")
    ulo_d = nc.dram_tensor("ulo", [P, KI * KH * P], f8, kind="ExternalInput")
    dhi_d = nc.dram_tensor("dhi", [P, KI2 * H], f8, kind="ExternalInput")
    dlo_d = nc.dram_tensor("dlo", [P, KI2 * H], f8, kind="ExternalInput")
    out_d = nc.dram_tensor("out", [P, KH * C], bf16, kind="ExternalOutput")

    g3 = ghi_d[:].rearrange("p (i k q) -> p i k q", i=KI, k=KH)
    gl3 = glo_d[:].rearrange("p (i k q) -> p i k q", i=KI, k=KH)
    u3 = uhi_d[:].rearrange("p (i k q) -> p i k q", i=KI, k=KH)
    ul3 = ulo_d[:].rearrange("p (i k q) -> p i k q", i=KI, k=KH)
    x3h = xhi_d[:].rearrange("p (k c) -> p k c", k=KH)
    x3l = xlo_d[:].rearrange("p (k c) -> p k c", k=KH)
    o3 = out_d[:].rearrange("p (k c) -> p k c", k=KH)

    with ExitStack() as ctx:
        tc = ctx.enter_context(tile.TileContext(nc))
        wpool = ctx.enter_context(tc.tile_pool(name="w", bufs=1))
        ipool = ctx.enter_context(tc.tile_pool(name="inter", bufs=1))
        spool = ctx.enter_context(tc.tile_pool(name="stage", bufs=1))
        ppool = ctx.enter_context(tc.tile_pool(name="psum", bufs=1, space="PSUM"))
        ppool2 = ctx.enter_context(tc.tile_pool(name="psum2", bufs=2, space="PSUM"))

        # weights resident: every load writes a fresh region
        ghi_sb = wpool.tile([P, KI, KH, P], f8, name="ghi_sb")
        glo_sb = wpool.tile([P, KI, KH, P], f8, name="glo_sb")
        uhi_sb = wpool.tile([P, KI, KH, P], f8, name="uhi_sb")
        ulo_sb = wpool.tile([P, KI, KH, P], f8, name="ulo_sb")
        dhi_sb = wpool.tile([P, KI2, H], f8, name="dhi_sb")
        dlo_sb = wpool.tile([P, KI2, H], f8, name="dlo_sb")
        sw_dmas = []
        for im in range(KI):
            sw_dmas.append(nc.gpsimd.dma_start(ghi_sb[:, im], g3[:, im]))
            sw_dmas.append(nc.gpsimd.dma_start(glo_sb[:, im], gl3[:, im]))
            sw_dmas.append(nc.gpsimd.dma_start(uhi_sb[:, im], u3[:, im]))
            sw_dmas.append(nc.gpsimd.dma_start(ulo_sb[:, im], ul3[:, im]))
        sw_dmas.append(nc.gpsimd.dma_start(
            dhi_sb[:], dhi_d[:].rearrange("p (i h) -> p i h", i=KI2)))
        sw_dmas.append(nc.gpsimd.dma_start(
            dlo_sb[:], dlo_d[:].rearrange("p (i h) -> p i h", i=KI2)))

        # x planes and out share one hand-allocated slab (x is dead after
        # phase 1; both hazard directions resolve through the PE semaphore).
        # The slab is split into four quarter tensors with matching offsets
        # on the x and out sides, so each out write/store aliases exactly
        # ONE x DMA (hazard tracking is tensor-granular).
        xo_arena = nc.alloc_sbuf_tensor("xo_arena", [P, KH * C], bf16)
        xo_off = nc.lookup_mloc(xo_arena).addr
        QB = KH // 4 * 2 * C  # quarter size in bytes (4 bf16 out rows)
        xh_sb = nc.alloc_sbuf_tensor_at("xh_sb", [P, KH, C], f8, offset=xo_off)[:]
        xl_sb = nc.alloc_sbuf_tensor_at(
            "xl_sb", [P, KH, C], f8, offset=xo_off + KH * C
        )[:]
        out_q = [
            nc.alloc_sbuf_tensor_at(
                f"outq{i}", [P, KH // 4, C], bf16, offset=xo_off + i * QB
            )[:]
            for i in range(4)
        ]
        x_dmas = [
            nc.sync.dma_start(xh_sb[:], x3h),
            nc.sync.dma_start(xl_sb[:], x3l),
        ]

        def xsl(plane, k, o, w):
            # x operand slice for k-subtile pair (k, k+1) of a plane
            t = xh_sb if plane == 0 else xl_sb
            return t[:, k : k + 2, o : o + w]

        ivl_sb = ipool.tile([P, KI, 2, C], f8, name="ivl_sb")
        # double-depth rotating staging (im%2, chunk): a slot's DVE readers
        # from round im-2 are absorbed into the PE proc by a per-im fence
        # that never stalls (two full rounds of PE work in between), so the
        # ACT rewrites and DVE ops carry no cross-WAR waits of their own
        t1_sb = spool.tile([P, 2, NJ, 512], bf16, name="t1_sb")
        v_sb = spool.tile([P, 2, NJ, 512], bf16, name="v_sb")
        # fence scratch: never written by real producers, so fence
        # reads/writes of their disjoint per-round columns carry no deps
        # beyond the explicit edges. Wait dedup is exact-instruction per
        # engine, so each fence lists the precise producer set (one
        # semaphore) it absorbs for its queue.
        fence_t = spool.tile([P, 64], bf16, name="fence_t")
        if _INIT_FENCE:
            nc.vector.memset(fence_t[:], 0.0)
        last_tt = [None]
        dve_by_im = {}
        act_by_im = {}

        # ---- phase 1: interT = silu(gateT) * upT, requantized hi/lo ----
        def chain24(psum, whi, wlo, im, o, w):
            n = 0
            for k in range(0, KH, 2):
                nc.tensor.matmul(
                    psum[:, :w], whi[:, im, k : k + 2], xsl(0, k, o, w),
                    start=(n == 0), stop=False, perf_mode=DR)
                n += 1
            for k in range(0, KH, 2):
                nc.tensor.matmul(
                    psum[:, :w], whi[:, im, k : k + 2], xsl(1, k, o, w),
                    start=False, stop=False, perf_mode=DR)
            for k in range(0, KH, 2):
                nc.tensor.matmul(
                    psum[:, :w], wlo[:, im, k : k + 2], xsl(0, k, o, w),
                    start=False, stop=(k == KH - 2), perf_mode=DR)

        for im in range(KI):
            s = im % 2
            if im >= 2:
                # absorb round im-2's exact producer sets into each queue:
                # af takes its DVE ops (one DVE sem), af2 its ACT ops (own
                # sem), df its DVE ops for the DVE queue. The staging
                # slots' WAR/WAW hazards this round then dedup away and
                # every real ACT/DVE instruction keeps a single wait.
                ca = 16 + 2 * (im - 2)
                af = nc.scalar.copy(
                    fence_t[:1, ca : ca + 2], fence_t[:1, ca : ca + 2])
                for bi in dve_by_im[im - 2]:
                    add_dep_helper(af.ins, bi.ins, sync=True,
                                   reason="act-dve slot fence")
                ca += 20
                af2 = nc.scalar.copy(
                    fence_t[:1, ca : ca + 2], fence_t[:1, ca : ca + 2])
                for bi in act_by_im[im - 2]:
                    add_dep_helper(af2.ins, bi.ins, sync=True,
                                   reason="act-act slot fence")
                # df's WAW against the previous df lands on the same DVE
                # semaphore as its edges, so one fixed column suffices
                df = nc.vector.tensor_copy(
                    fence_t[:1, 56:58], fence_t[:1, 56:58])
                for bi in dve_by_im[im - 2]:
                    add_dep_helper(df.ins, bi.ins, sync=True,
                                   reason="dve slot fence")
            pg = [
                ppool.tile([P, w], f32, tag=f"a{j}", name=f"pg{j}")
                for j, (_, w) in enumerate(chunks)
            ]
            pu = [
                ppool.tile([P, w], f32, tag=f"b{j}", name=f"pu{j}")
                for j, (_, w) in enumerate(chunks)
            ]
            for j, (o, w) in enumerate(chunks):
                chain24(pg[j], ghi_sb, glo_sb, im, o, w)
                chain24(pu[j], uhi_sb, ulo_sb, im, o, w)
            dve_by_im[im] = []
            act_by_im[im] = []
            for j, (o, w) in enumerate(chunks):
                # t1 = silu(g) (scale folds the fp8 scaling out before the
                # nonlinearity); v = u * S_I, then in-place v = inter * S_I
                a1 = nc.scalar.activation(
                    t1_sb[:, s, j, :w], pg[j][:],
                    mybir.ActivationFunctionType.Silu, scale=1.0 / (S_W * S_X))
                a2 = nc.scalar.activation(
                    v_sb[:, s, j, :w], pu[j][:],
                    mybir.ActivationFunctionType.Copy, scale=S_I / (S_W * S_X))
                # DVE chain: mul reads two ACT products (one collapsed sem),
                # copy/sub read only DVE-produced data (self sem)
                d1 = nc.vector.tensor_mul(
                    v_sb[:, s, j, :w], t1_sb[:, s, j, :w], v_sb[:, s, j, :w])
                d2 = nc.vector.tensor_copy(
                    ivl_sb[:, im, 0, o : o + w], v_sb[:, s, j, :w])
                d3 = nc.vector.tensor_sub(
                    ivl_sb[:, im, 1, o : o + w],
                    v_sb[:, s, j, :w],
                    ivl_sb[:, im, 0, o : o + w],
                )
                last_tt[0] = d3
                act_by_im[im] += [a1, a2]
                dve_by_im[im] += [d1, d2, d3]

        # ---- phase 2: outT = down_w @ interT ----
        # absorb every x-DMA completion into the ACT proc and the SP queue
        # (exact-instruction dedup: later out writes/stores on the aliased
        # slab then drop their WAW-vs-x deps and carry only PE/ACT). The
        # ACT fences use dep-free fence columns; the SP dummies read tiny
        # x slices whose RAW lands on the same DMA they absorb.
        # each absorber lowers to one wait: the ACT fences take the x DMAs
        # for the ACT queue (out copies then dedup their WAW-vs-x), the SP
        # dummy stores take them for the SP queue (out stores dedup), and
        # two more ACT fences take the dummy stores themselves (the out
        # copies' WAR against the dummies' slab reads then dedups too)
        scr_d = nc.dram_tensor("scr", [2, 16], f8)
        dummy_stores = []
        for fi, (xd, src) in enumerate(
            zip(x_dmas, [xh_sb[0:1, 0, 0:16], xl_sb[0:1, 0, 0:16]])
        ):
            fence = nc.scalar.copy(
                fence_t[:1, fi * 2 : fi * 2 + 2],
                fence_t[:1, fi * 2 : fi * 2 + 2])
            add_dep_helper(fence.ins, xd.ins, sync=True, reason="x lane fence")
            ds = nc.sync.dma_start(scr_d[fi : fi + 1], src)
            add_dep_helper(ds.ins, xd.ins, sync=True, reason="x sp fence")
            dummy_stores.append(ds)
            dsf = nc.scalar.copy(
                fence_t[:1, 4 + fi * 2 : 6 + fi * 2],
                fence_t[:1, 4 + fi * 2 : 6 + fi * 2])
            add_dep_helper(dsf.ins, ds.ins, sync=True, reason="ds act fence")
        # absorb the last TT's DVE tick into the PE proc so phase-2 matmuls
        # wait only on their dw DMA lane
        pe_fence = nc.tensor.ldweights(ghi_sb[:, 0, 0, 0:1])
        add_dep_helper(pe_fence.ins, last_tt[0].ins, sync=True, reason="pe fence")
        hw_dmas = list(x_dmas) + dummy_stores
        tail_insts = []
        last_mm = []
        for hm in range(KH):
            po = [
                ppool2.tile([P, w], f32, tag=f"c{j}", name=f"po{j}")
                for j, (_, w) in enumerate(chunks)
            ]
            for j, (o, w) in enumerate(chunks):
                hs = hm * P
                n = 0
                for k in range(0, KI - 1, 2):
                    nc.tensor.matmul(
                        po[j][:, :w], dhi_sb[:, k : k + 2, hs : hs + P],
                        ivl_sb[:, k : k + 2, 0, o : o + w],
                        start=(n == 0), stop=False, perf_mode=DR)
                    n += 1
                for k in range(0, KI - 1, 2):
                    nc.tensor.matmul(
                        po[j][:, :w], dhi_sb[:, k : k + 2, hs : hs + P],
                        ivl_sb[:, k : k + 2, 1, o : o + w],
                        start=False, stop=False, perf_mode=DR)
                for k in range(0, KI - 1, 2):
                    nc.tensor.matmul(
                        po[j][:, :w], dlo_sb[:, k : k + 2, hs : hs + P],
                        ivl_sb[:, k : k + 2, 0, o : o + w],
                        start=False, stop=False, perf_mode=DR)
                # odd last subtile: its hi/lo planes are adjacent in ivl,
                # paired against the duplicated stationary slot KI..KI+1
                nc.tensor.matmul(
                    po[j][:, :w], dhi_sb[:, KI - 1 : KI + 1, hs : hs + P],
                    ivl_sb[:, KI - 1, 0:2, o : o + w],
                    start=False, stop=False, perf_mode=DR)
                last_mm.append(nc.tensor.matmul(
                    po[j][:, :w], dlo_sb[:, KI - 1 : KI + 1, hs : hs + P],
                    ivl_sb[:, KI - 1, 0:2, o : o + w],
                    start=False, stop=True, perf_mode=DR))
                del last_mm[:-1]
            q, qr = hm // 4, hm % 4
            for j, (o, w) in enumerate(chunks):
                tail_insts.append(nc.scalar.activation(
                    out_q[q][:, qr, o : o + w], po[j][:],
                    mybir.ActivationFunctionType.Copy, scale=1.0 / (S_W * S_I)))
            # 4 quarter stores: with the 4 x loads that is exactly 8 SP
            # DMAs, so each lands on a fresh HW lane (no ring wait)
            if qr == 3:
                hw_dmas.append(nc.sync.dma_start(
                    o3[:, hm - 3 : hm + 1, :], out_q[q][:]
                ))

        # pre-drain: absorb every proc's final tick into the SP sequencer
        # one sync edge at a time, so the kernel-tail drain's waits (which
        # would exceed the instruction's wait slots) are all elided
        for insts in (sw_dmas, hw_dmas, [last_tt[0]], tail_insts[-2:], last_mm[-1:]):
            for bi in insts:
                if bi is None:
                    continue
                nop = nc.sync.nop()
                add_dep_helper(nop.ins, bi.ins, sync=True, reason="pre-drain")

    return nc


def _split_fp8(a, scale):
    """Split scale*a into hi/lo float8_e4m3 planes (as fp8 arrays)."""
    import ml_dtypes

    f8 = ml_dtypes.float8_e4m3
    v = (a * scale).astype(np.float32)
    hi = v.astype(f8)
    lo = (v - hi.astype(np.float32)).astype(f8)
    return hi, lo


def kernel(hidden_states, top_k_index, top_k_weights, gate_w, up_w, down_w):
    import ml_dtypes
    from concourse.bass_utils import run_bass_kernel_spmd

    hs = np.ascontiguousarray(np.asarray(hidden_states, dtype=np.float32))
    tki = np.asarray(top_k_index)
    tkw = np.asarray(top_k_weights, dtype=np.float32)
    gw = np.asarray(gate_w, dtype=np.float32)
    uw = np.asarray(up_w, dtype=np.float32)
    dw = np.asarray(down_w, dtype=np.float32)

    T, H = hs.shape
    E, I, _ = gw.shape
    P = 128
    KH = H // P
    KI = I // P
    KI2 = KI + 1

    tok_lists, w_lists = [], []
    for e in range(E):
        mask = tki == e
        toks = np.nonzero(mask.any(axis=1))[0]
        w = (tkw * mask).sum(axis=1)[toks].astype(np.float32)
        tok_lists.append(toks)
        w_lists.append(w)

    # capacity: smallest multiple of 16 covering the busiest expert
    # (1008 for the balanced T=4096, K=2, E=8 regime); experts with more
    # tokens spill into additional SPMD rounds
    maxn = max(len(t) for t in tok_lists)
    C = min(1024, -(-maxn // 16) * 16)
    n_rounds = max(1, -(-maxn // C))

    def pack_gu(a):  # [I, H] fp8 -> [P, KI*KH*P] im-block-major
        # column block im of a.T[H, I] as [P, KH, 128], contiguous per block
        b = a.T.reshape(KH, P, KI, P).transpose(1, 2, 0, 3)  # p, im, kh, q
        return np.ascontiguousarray(b.reshape(P, KI * KH * P))

    def pack_d(a):  # [H, I] fp8 -> [P, KI2*H] with duplicated last subtile
        b = a.T.reshape(KI, P, H).transpose(1, 0, 2)  # p, ki, h
        b = np.concatenate([b, b[:, KI - 1 : KI]], axis=1)  # dup slot
        return np.ascontiguousarray(b.reshape(P, KI2 * H))

    wmaps = []
    for e in range(E):
        ghi, glo = _split_fp8(gw[e], S_W)
        uhi, ulo = _split_fp8(uw[e], S_W)
        dhi, dlo = _split_fp8(dw[e], S_W)
        wmaps.append({
            "ghi": pack_gu(ghi), "glo": pack_gu(glo),
            "uhi": pack_gu(uhi), "ulo": pack_gu(ulo),
            "dhi": pack_d(dhi), "dlo": pack_d(dlo),
        })

    nc = _build_bass(C, H, I)
    out = np.zeros((T, H), np.float32)
    global _last_results
    for r in range(n_rounds):
        in_maps = []
        for e in range(E):
            toks = tok_lists[e][r * C : (r + 1) * C]
            xT = np.zeros((H, C), np.float32)
            xT[:, : len(toks)] = hs[toks].T
            xhi, xlo = _split_fp8(xT, S_X)

            def pack_x(a):  # [H, C] -> [P, KH*C]
                b = a.reshape(KH, P, C).transpose(1, 0, 2)
                return np.ascontiguousarray(b.reshape(P, KH * C))

            in_maps.append({"xhi": pack_x(xhi), "xlo": pack_x(xlo), **wmaps[e]})
        res = run_bass_kernel_spmd(nc, in_maps, core_ids=list(range(E)))
        _last_results = res
        for e in range(E):
            toks = tok_lists[e][r * C : (r + 1) * C]
            n = len(toks)
            if n == 0:
                continue
            o = np.asarray(res.results[e]["out"]).astype(np.float32)  # [P, KH*C]
            outT = o.reshape(P, KH, C).transpose(1, 0, 2).reshape(H, C)
            out[toks] += w_lists[e][r * C : r * C + n, None] * outT[:, :n].T
    return out


# revision 61
# speedup vs baseline: 1.4471x; 1.0508x over previous
"""MoE SwiGLU experts kernel for Trainium2 (8 NeuronCores, expert-parallel).

Each core owns one expert e. Host does the dispatch (gathers tokens whose
top-k includes e, dedups with summed combine weights), splits every matmul
operand into two fp8-e4m3 planes (hi = fp8(v), lo = fp8(v - hi), with
power-of-two scales so the uniform-[-1/sqrt(H)] weights clear the fp8
denormal floor), and pre-swizzles everything to partition-major layouts.
Device computes the SwiGLU MLP with fp8 DoubleRow matmuls: each
instruction contracts two 128-row k-subtiles, and the three significant
cross-products (hi*hi, hi*lo, lo*hi) recover ~bf16 accuracy at half the
PE cycles of bf16:

    gateT = gate_w[e] @ x_eT          # [I, C]  (contract H, 24 instrs/chain)
    upT   = up_w[e]   @ x_eT          # [I, C]
    interT = silu(gateT) * upT        # [I, C]  requantized to fp8 hi/lo
    outT  = down_w[e]  @ interT       # [H, C]  (contract I, 17 instrs/chain)

The intermediate is stored hi/lo-interleaved [P, KI, 2, C] so phase-2
moving operands pair adjacent k-subtiles (planes strided) and the odd
11th subtile pairs its own hi/lo planes against a duplicated stationary
slot. Host scatter-adds w * outT.T rows into the [T, H] output.

Hardware sync-wait slots per instruction are scarce (walrus rejects
kernels that need too many), so the structure keeps every instruction's
dependency fan-in tiny:
- all weight planes and the token slab are fully SBUF-resident and
  written once, so their DMAs carry no WAR/WAW waits;
- the evac pipeline alternates ACT/DVE so same-engine program order
  subsumes most deps; slot-rotating staging is rewritten only after its
  single cross-engine reader ran;
- output stores are batched so each lands on a fresh HW DMA lane.
"""

import numpy as np

# power-of-two scales: keep fp8 operands out of the denormal floor and
# under the e4m3 max (240); all folded back via ACT scale params
S_W = float(2.0**12)
S_X = float(2.0**4)
S_I = float(2.0**3)

# CoreSim-only: initialize the fence scratch so the interpreter's
# uninitialized-read check passes. The device build must NOT set this -
# fence reads of unwritten scratch are intentionally dependency-free.
_INIT_FENCE = False


def _build_bass(C: int, H: int, I: int):
    from contextlib import ExitStack

    import concourse.bass as bass
    import concourse.mybir as mybir
    import concourse.tile as tile
    from concourse.tile import add_dep_helper

    f32 = mybir.dt.float32
    bf16 = mybir.dt.bfloat16
    f8 = mybir.dt.float8e4
    P = 128
    KH = H // P  # 16
    KI = I // P  # 11
    KI2 = KI + 1  # down stationary gets a duplicate of the odd last subtile
    DR = mybir.MatmulPerfMode.DoubleRow

    chunks = []
    off = 0
    while off < C:
        w = min(512, C - off)
        chunks.append((off, w))
        off += w
    NJ = len(chunks)

    nc = bass.Bass(dynamic_dma_scratch_size=8192)
    # all inputs pre-swizzled on host to [P, ...] partition-major layouts
    xhi_d = nc.dram_tensor("xhi", [P, KH * C], f8, kind="ExternalInput")
    xlo_d = nc.dram_tensor("xlo", [P, KH * C], f8, kind="ExternalInput")
    ghi_d = nc.dram_tensor("ghi", [P, KI * KH * P], f8, kind="ExternalInput")
    glo_d = nc.dram_tensor("glo", [P, KI * KH * P], f8, kind="ExternalInput")
    uhi_d = nc.dram_tensor("uhi", [P, KI * KH * P], f8, kind="ExternalInput")
    ulo_d = nc.dram_tensor("ulo", [P, KI * KH * P], f8, kind="ExternalInput")
    dhi_d = nc.dram_tensor("dhi", [P, KI2 * H], f8, kind="ExternalInput")
    dlo_d = nc.dram_tensor("dlo", [P, KI2 * H], f8, kind="ExternalInput")
    out_d = nc.dram_tensor("out", [P, KH * C], bf16, kind="ExternalOutput")

    g3 = ghi_d[:].rearrange("p (i k q) -> p i k q", i=KI, k=KH)
    gl3 = glo_d[:].rearrange("p (i k q) -> p i k q", i=KI, k=KH)
    u3 = uhi_d[:].rearrange("p (i k q) -> p i k q", i=KI, k=KH)
    ul3 = ulo_d[:].rearrange("p (i k q) -> p i k q", i=KI, k=KH)
    x3h = xhi_d[:].rearrange("p (k c) -> p k c", k=KH)
    x3l = xlo_d[:].rearrange("p (k c) -> p k c", k=KH)
    o3 = out_d[:].rearrange("p (k c) -> p k c", k=KH)

    with ExitStack() as ctx:
        tc = ctx.enter_context(tile.TileContext(nc))
        wpool = ctx.enter_context(tc.tile_pool(name="w", bufs=1))
        ipool = ctx.enter_context(tc.tile_pool(name="inter", bufs=1))
        spool = ctx.enter_context(tc.tile_pool(name="stage", bufs=1))
        ppool = ctx.enter_context(tc.tile_pool(name="psum", bufs=1, space="PSUM"))
        ppool2 = ctx.enter_context(tc.tile_pool(name="psum2", bufs=2, space="PSUM"))

        # weights resident: every load writes a fresh region
        ghi_sb = wpool.tile([P, KI, KH, P], f8, name="ghi_sb")
        glo_sb = wpool.tile([P, KI, KH, P], f8, name="glo_sb")
        uhi_sb = wpool.tile([P, KI, KH, P], f8, name="uhi_sb")
        ulo_sb = wpool.tile([P, KI, KH, P], f8, name="ulo_sb")
        dhi_sb = wpool.tile([P, KI2, H], f8, name="dhi_sb")
        dlo_sb = wpool.tile([P, KI2, H], f8, name="dlo_sb")
        sw_dmas = []

        def wblk(im):
            sw_dmas.append(nc.gpsimd.dma_start(ghi_sb[:, im], g3[:, im]))
            sw_dmas.append(nc.gpsimd.dma_start(glo_sb[:, im], gl3[:, im]))
            sw_dmas.append(nc.gpsimd.dma_start(uhi_sb[:, im], u3[:, im]))
            sw_dmas.append(nc.gpsimd.dma_start(ulo_sb[:, im], ul3[:, im]))

        # x pieces as separate write-once tensors (hazard tracking is
        # tensor-granular, so each matmul deps only its own piece's DMA);
        # out is a small ring that never aliases x, so out writes/stores
        # carry no x hazards at all
        xp = [
            wpool.tile([P, KH // 4, C], f8, name=f"xp{i}") for i in range(8)
        ]
        out_r = wpool.tile([P, KH // 4, C], bf16, name="out_r")

        # stream order is what the (serialized) DMA engine model follows:
        # first gate weights + the xh pieces that unlock P1/P3 work, then
        # up weights + the xl pieces, then the weight tail. x pieces ride
        # the otherwise idle ACT/DVE HWDGE rings.
        wblk(0)
        R = KH // 4  # x piece rows
        x_dmas = []
        for i in range(4):
            x_dmas.append(
                nc.gpsimd.dma_start(xp[i][:], x3h[:, i * R : (i + 1) * R]))
        wblk(1)
        for i in range(4):
            x_dmas.append(
                nc.gpsimd.dma_start(xp[4 + i][:], x3l[:, i * R : (i + 1) * R]))
        for im in range(2, KI):
            wblk(im)
        sw_dmas.append(nc.gpsimd.dma_start(
            dhi_sb[:], dhi_d[:].rearrange("p (i h) -> p i h", i=KI2)))
        sw_dmas.append(nc.gpsimd.dma_start(
            dlo_sb[:], dlo_d[:].rearrange("p (i h) -> p i h", i=KI2)))

        def xsl(plane, k, o, w):
            # x operand slice for k-subtile pair (k, k+1) of a plane
            t = xp[plane * 4 + k // R]
            kk = k % R
            return t[:, kk : kk + 2, o : o + w]

        ivl_sb = ipool.tile([P, KI, 2, C], f8, name="ivl_sb")
        # double-depth rotating staging (im%2, chunk): a slot's DVE readers
        # from round im-2 are absorbed into the PE proc by a per-im fence
        # that never stalls (two full rounds of PE work in between), so the
        # ACT rewrites and DVE ops carry no cross-WAR waits of their own
        t1_sb = spool.tile([P, 2, NJ, 512], bf16, name="t1_sb")
        v_sb = spool.tile([P, 2, NJ, 512], bf16, name="v_sb")
        # fence scratch: never written by real producers, so fence
        # reads/writes of their disjoint per-round columns carry no deps
        # beyond the explicit edges. Wait dedup is exact-instruction per
        # engine, so each fence lists the precise producer set (one
        # semaphore) it absorbs for its queue.
        fence_t = spool.tile([P, 64], bf16, name="fence_t")
        if _INIT_FENCE:
            nc.vector.memset(fence_t[:], 0.0)
        last_tt = [None]
        dve_by_im = {}
        act_by_im = {}

        # ---- phase 1: interT = silu(gateT) * upT, requantized hi/lo ----
        def chain24(psum, whi, wlo, im, o, w):
            # hi-plane products grouped by x piece (so each arriving piece
            # unlocks work during the prologue), lo-plane products last
            first = True
            for k in range(0, KH, 2):
                nc.tensor.matmul(
                    psum[:, :w], whi[:, im, k : k + 2], xsl(0, k, o, w),
                    start=first, stop=False, perf_mode=DR)
                first = False
                nc.tensor.matmul(
                    psum[:, :w], wlo[:, im, k : k + 2], xsl(0, k, o, w),
                    start=False, stop=False, perf_mode=DR)
            for k in range(0, KH, 2):
                nc.tensor.matmul(
                    psum[:, :w], whi[:, im, k : k + 2], xsl(1, k, o, w),
                    start=False, stop=(k == KH - 2), perf_mode=DR)

        for im in range(KI):
            s = im % 2
            if im >= 2:
                # absorb round im-2's exact producer sets into each queue:
                # af takes its DVE ops (one DVE sem), af2 its ACT ops (own
                # sem), df its DVE ops for the DVE queue. The staging
                # slots' WAR/WAW hazards this round then dedup away and
                # every real ACT/DVE instruction keeps a single wait.
                ca = 16 + 2 * (im - 2)
                af = nc.scalar.copy(
                    fence_t[:1, ca : ca + 2], fence_t[:1, ca : ca + 2])
                for bi in dve_by_im[im - 2]:
                    add_dep_helper(af.ins, bi.ins, sync=True,
                                   reason="act-dve slot fence")
                ca += 20
                af2 = nc.scalar.copy(
                    fence_t[:1, ca : ca + 2], fence_t[:1, ca : ca + 2])
                for bi in act_by_im[im - 2]:
                    add_dep_helper(af2.ins, bi.ins, sync=True,
                                   reason="act-act slot fence")
                # df's WAW against the previous df lands on the same DVE
                # semaphore as its edges, so one fixed column suffices
                df = nc.vector.tensor_copy(
                    fence_t[:1, 56:58], fence_t[:1, 56:58])
                for bi in dve_by_im[im - 2]:
                    add_dep_helper(df.ins, bi.ins, sync=True,
                                   reason="dve slot fence")
            pg = [
                ppool.tile([P, w], f32, tag=f"a{j}", name=f"pg{j}")
                for j, (_, w) in enumerate(chunks)
            ]
            pu = [
                ppool.tile([P, w], f32, tag=f"b{j}", name=f"pu{j}")
                for j, (_, w) in enumerate(chunks)
            ]
            for j, (o, w) in enumerate(chunks):
                chain24(pg[j], ghi_sb, glo_sb, im, o, w)
                chain24(pu[j], uhi_sb, ulo_sb, im, o, w)
            dve_by_im[im] = []
            act_by_im[im] = []
            for j, (o, w) in enumerate(chunks):
                # t1 = silu(g) (scale folds the fp8 scaling out before the
                # nonlinearity); v = u * S_I, then in-place v = inter * S_I
                a1 = nc.scalar.activation(
                    t1_sb[:, s, j, :w], pg[j][:],
                    mybir.ActivationFunctionType.Silu, scale=1.0 / (S_W * S_X))
                a2 = nc.scalar.activation(
                    v_sb[:, s, j, :w], pu[j][:],
                    mybir.ActivationFunctionType.Copy, scale=S_I / (S_W * S_X))
                # DVE chain: mul reads two ACT products (one collapsed sem),
                # copy/sub read only DVE-produced data (self sem)
                d1 = nc.vector.tensor_mul(
                    v_sb[:, s, j, :w], t1_sb[:, s, j, :w], v_sb[:, s, j, :w])
                d2 = nc.vector.tensor_copy(
                    ivl_sb[:, im, 0, o : o + w], v_sb[:, s, j, :w])
                d3 = nc.vector.tensor_sub(
                    ivl_sb[:, im, 1, o : o + w],
                    v_sb[:, s, j, :w],
                    ivl_sb[:, im, 0, o : o + w],
                )
                last_tt[0] = d3
                act_by_im[im] += [a1, a2]
                dve_by_im[im] += [d1, d2, d3]

        # ---- phase 2: outT = down_w @ interT ----
        # absorb every x-DMA completion into the ACT proc and the SP queue
        # (exact-instruction dedup: later out writes/stores on the aliased
        # slab then drop their WAW-vs-x deps and carry only PE/ACT). The
        # ACT fences use dep-free fence columns; the SP dummies read tiny
        # x slices whose RAW lands on the same DMA they absorb.

        # absorb the last TT's DVE tick into the PE proc so phase-2 matmuls
        # wait only on their dw DMA lane
        pe_fence = nc.tensor.ldweights(ghi_sb[:, 0, 0, 0:1])
        add_dep_helper(pe_fence.ins, last_tt[0].ins, sync=True, reason="pe fence")
        hw_dmas = list(x_dmas)
        tail_insts = []
        last_mm = []
        for hm in range(KH):
            po = [
                ppool2.tile([P, w], f32, tag=f"c{j}", name=f"po{j}")
                for j, (_, w) in enumerate(chunks)
            ]
            for j, (o, w) in enumerate(chunks):
                hs = hm * P
                n = 0
                for k in range(0, KI - 1, 2):
                    nc.tensor.matmul(
                        po[j][:, :w], dhi_sb[:, k : k + 2, hs : hs + P],
                        ivl_sb[:, k : k + 2, 0, o : o + w],
                        start=(n == 0), stop=False, perf_mode=DR)
                    n += 1
                for k in range(0, KI - 1, 2):
                    nc.tensor.matmul(
                        po[j][:, :w], dhi_sb[:, k : k + 2, hs : hs + P],
                        ivl_sb[:, k : k + 2, 1, o : o + w],
                        start=False, stop=False, perf_mode=DR)
                for k in range(0, KI - 1, 2):
                    nc.tensor.matmul(
                        po[j][:, :w], dlo_sb[:, k : k + 2, hs : hs + P],
                        ivl_sb[:, k : k + 2, 0, o : o + w],
                        start=False, stop=False, perf_mode=DR)
                # odd last subtile: its hi/lo planes are adjacent in ivl,
                # paired against the duplicated stationary slot KI..KI+1
                nc.tensor.matmul(
                    po[j][:, :w], dhi_sb[:, KI - 1 : KI + 1, hs : hs + P],
                    ivl_sb[:, KI - 1, 0:2, o : o + w],
                    start=False, stop=False, perf_mode=DR)
                last_mm.append(nc.tensor.matmul(
                    po[j][:, :w], dlo_sb[:, KI - 1 : KI + 1, hs : hs + P],
                    ivl_sb[:, KI - 1, 0:2, o : o + w],
                    start=False, stop=True, perf_mode=DR))
                del last_mm[:-1]
            q, qr = hm // (KH // 4), hm % (KH // 4)
            if qr == 0 and q >= 1:
                # ring handoff: absorb the previous quarter's copies (ACT
                # self sem) and its two pair stores (one fence per DMA
                # semaphore) so this quarter's copies keep their single
                # PE wait
                f3a = nc.scalar.copy(fence_t[:1, 0:2], fence_t[:1, 0:2])
                for bi in tail_insts[-2 * (KH // 4):]:
                    add_dep_helper(f3a.ins, bi.ins, sync=True,
                                   reason="ring copy fence")
                for si, st in enumerate(hw_dmas[-2:]):
                    ca = 2 + 4 * (q - 1) + 2 * si
                    f3b = nc.scalar.copy(
                        fence_t[:1, ca : ca + 2], fence_t[:1, ca : ca + 2])
                    add_dep_helper(f3b.ins, st.ins, sync=True,
                                   reason="ring store fence")
            for j, (o, w) in enumerate(chunks):
                tail_insts.append(nc.scalar.activation(
                    out_r[:, qr, o : o + w], po[j][:],
                    mybir.ActivationFunctionType.Copy, scale=1.0 / (S_W * S_I)))
            # 8 pair stores on the SP ring - exactly 8 DMAs, each on a
            # fresh HW lane (x rides the Pool SWDGE ring)
            if qr % 2 == 1:
                hw_dmas.append(nc.sync.dma_start(
                    o3[:, hm - 1 : hm + 1, :], out_r[:, qr - 1 : qr + 1]
                ))

        # pre-drain: absorb every proc's final tick into the SP sequencer
        # one sync edge at a time, so the kernel-tail drain's waits (which
        # would exceed the instruction's wait slots) are all elided
        for insts in (sw_dmas, hw_dmas, [last_tt[0]], tail_insts[-2:], last_mm[-1:]):
            for bi in insts:
                if bi is None:
                    continue
                nop = nc.sync.nop()
                add_dep_helper(nop.ins, bi.ins, sync=True, reason="pre-drain")

    return nc


def _split_fp8(a, scale):
    """Split scale*a into hi/lo float8_e4m3 planes (as fp8 arrays)."""
    import ml_dtypes

    f8 = ml_dtypes.float8_e4m3
    v = (a * scale).astype(np.float32)
    hi = v.astype(f8)
    lo = (v - hi.astype(np.float32)).astype(f8)
    return hi, lo


def kernel(hidden_states, top_k_index, top_k_weights, gate_w, up_w, down_w):
    import ml_dtypes
    from concourse.bass_utils import run_bass_kernel_spmd

    hs = np.ascontiguousarray(np.asarray(hidden_states, dtype=np.float32))
    tki = np.asarray(top_k_index)
    tkw = np.asarray(top_k_weights, dtype=np.float32)
    gw = np.asarray(gate_w, dtype=np.float32)
    uw = np.asarray(up_w, dtype=np.float32)
    dw = np.asarray(down_w, dtype=np.float32)

    T, H = hs.shape
    E, I, _ = gw.shape
    P = 128
    KH = H // P
    KI = I // P
    KI2 = KI + 1

    tok_lists, w_lists = [], []
    for e in range(E):
        mask = tki == e
        toks = np.nonzero(mask.any(axis=1))[0]
        w = (tkw * mask).sum(axis=1)[toks].astype(np.float32)
        tok_lists.append(toks)
        w_lists.append(w)

    # capacity: smallest multiple of 16 covering the busiest expert
    # (1008 for the balanced T=4096, K=2, E=8 regime); experts with more
    # tokens spill into additional SPMD rounds
    maxn = max(len(t) for t in tok_lists)
    C = min(1024, -(-maxn // 16) * 16)
    n_rounds = max(1, -(-maxn // C))

    def pack_gu(a):  # [I, H] fp8 -> [P, KI*KH*P] im-block-major
        # column block im of a.T[H, I] as [P, KH, 128], contiguous per block
        b = a.T.reshape(KH, P, KI, P).transpose(1, 2, 0, 3)  # p, im, kh, q
        return np.ascontiguousarray(b.reshape(P, KI * KH * P))

    def pack_d(a):  # [H, I] fp8 -> [P, KI2*H] with duplicated last subtile
        b = a.T.reshape(KI, P, H).transpose(1, 0, 2)  # p, ki, h
        b = np.concatenate([b, b[:, KI - 1 : KI]], axis=1)  # dup slot
        return np.ascontiguousarray(b.reshape(P, KI2 * H))

    wmaps = []
    for e in range(E):
        ghi, glo = _split_fp8(gw[e], S_W)
        uhi, ulo = _split_fp8(uw[e], S_W)
        dhi, dlo = _split_fp8(dw[e], S_W)
        wmaps.append({
            "ghi": pack_gu(ghi), "glo": pack_gu(glo),
            "uhi": pack_gu(uhi), "ulo": pack_gu(ulo),
            "dhi": pack_d(dhi), "dlo": pack_d(dlo),
        })

    nc = _build_bass(C, H, I)
    out = np.zeros((T, H), np.float32)
    global _last_results
    for r in range(n_rounds):
        in_maps = []
        for e in range(E):
            toks = tok_lists[e][r * C : (r + 1) * C]
            xT = np.zeros((H, C), np.float32)
            xT[:, : len(toks)] = hs[toks].T
            xhi, xlo = _split_fp8(xT, S_X)

            def pack_x(a):  # [H, C] -> [P, KH*C]
                b = a.reshape(KH, P, C).transpose(1, 0, 2)
                return np.ascontiguousarray(b.reshape(P, KH * C))

            in_maps.append({"xhi": pack_x(xhi), "xlo": pack_x(xlo), **wmaps[e]})
        res = run_bass_kernel_spmd(nc, in_maps, core_ids=list(range(E)))
        _last_results = res
        for e in range(E):
            toks = tok_lists[e][r * C : (r + 1) * C]
            n = len(toks)
            if n == 0:
                continue
            o = np.asarray(res.results[e]["out"]).astype(np.float32)  # [P, KH*C]
            outT = o.reshape(P, KH, C).transpose(1, 0, 2).reshape(H, C)
            out[toks] += w_lists[e][r * C : r * C + n, None] * outT[:, :n].T
    return out


# revision 64
# speedup vs baseline: 1.4595x; 1.0086x over previous
"""MoE SwiGLU experts kernel for Trainium2 (8 NeuronCores, expert-parallel).

Each core owns one expert e. Host does the dispatch (gathers tokens whose
top-k includes e, dedups with summed combine weights), splits every matmul
operand into two fp8-e4m3 planes (hi = fp8(v), lo = fp8(v - hi), with
power-of-two scales so the uniform-[-1/sqrt(H)] weights clear the fp8
denormal floor), and pre-swizzles everything to partition-major layouts.
Device computes the SwiGLU MLP with fp8 DoubleRow matmuls: each
instruction contracts two 128-row k-subtiles, and the three significant
cross-products (hi*hi, hi*lo, lo*hi) recover ~bf16 accuracy at half the
PE cycles of bf16:

    gateT = gate_w[e] @ x_eT          # [I, C]  (contract H, 24 instrs/chain)
    upT   = up_w[e]   @ x_eT          # [I, C]
    interT = silu(gateT) * upT        # [I, C]  requantized to fp8 hi/lo
    outT  = down_w[e]  @ interT       # [H, C]  (contract I, 17 instrs/chain)

The intermediate is stored hi/lo-interleaved [P, KI, 2, C] so phase-2
moving operands pair adjacent k-subtiles (planes strided) and the odd
11th subtile pairs its own hi/lo planes against a duplicated stationary
slot. Host scatter-adds w * outT.T rows into the [T, H] output.

Hardware sync-wait slots per instruction are scarce (walrus rejects
kernels that need too many), so the structure keeps every instruction's
dependency fan-in tiny:
- all weight planes and the token slab are fully SBUF-resident and
  written once, so their DMAs carry no WAR/WAW waits;
- the evac pipeline alternates ACT/DVE so same-engine program order
  subsumes most deps; slot-rotating staging is rewritten only after its
  single cross-engine reader ran;
- output stores are batched so each lands on a fresh HW DMA lane.
"""

import numpy as np

# power-of-two scales: keep fp8 operands out of the denormal floor and
# under the e4m3 max (240); all folded back via ACT scale params
S_W = float(2.0**12)
S_X = float(2.0**4)
S_I = float(2.0**3)

# CoreSim-only: initialize the fence scratch so the interpreter's
# uninitialized-read check passes. The device build must NOT set this -
# fence reads of unwritten scratch are intentionally dependency-free.
_INIT_FENCE = False


def _build_bass(C: int, H: int, I: int):
    from contextlib import ExitStack

    import concourse.bass as bass
    import concourse.mybir as mybir
    import concourse.tile as tile
    from concourse.tile import add_dep_helper

    f32 = mybir.dt.float32
    bf16 = mybir.dt.bfloat16
    f8 = mybir.dt.float8e4
    P = 128
    KH = H // P  # 16
    KI = I // P  # 11
    KI2 = KI + 1  # down stationary gets a duplicate of the odd last subtile
    DR = mybir.MatmulPerfMode.DoubleRow

    chunks = []
    off = 0
    while off < C:
        w = min(512, C - off)
        chunks.append((off, w))
        off += w
    NJ = len(chunks)

    nc = bass.Bass(dynamic_dma_scratch_size=8192)
    # all inputs pre-swizzled on host to [P, ...] partition-major layouts
    xhi_d = nc.dram_tensor("xhi", [P, KH * C], f8, kind="ExternalInput")
    xlo_d = nc.dram_tensor("xlo", [P, KH * C], f8, kind="ExternalInput")
    ghi_d = nc.dram_tensor("ghi", [P, KI * KH * P], f8, kind="ExternalInput")
    glo_d = nc.dram_tensor("glo", [P, KI * KH * P], f8, kind="ExternalInput")
    uhi_d = nc.dram_tensor("uhi", [P, KI * KH * P], f8, kind="ExternalInput")
    ulo_d = nc.dram_tensor("ulo", [P, KI * KH * P], f8, kind="ExternalInput")
    dhi_d = nc.dram_tensor("dhi", [P, KI2 * H], f8, kind="ExternalInput")
    dlo_d = nc.dram_tensor("dlo", [P, KI2 * H], f8, kind="ExternalInput")
    out_d = nc.dram_tensor("out", [P, KH * C], bf16, kind="ExternalOutput")

    g3 = ghi_d[:].rearrange("p (i k q) -> p i k q", i=KI, k=KH)
    gl3 = glo_d[:].rearrange("p (i k q) -> p i k q", i=KI, k=KH)
    u3 = uhi_d[:].rearrange("p (i k q) -> p i k q", i=KI, k=KH)
    ul3 = ulo_d[:].rearrange("p (i k q) -> p i k q", i=KI, k=KH)
    x3h = xhi_d[:].rearrange("p (k c) -> p k c", k=KH)
    x3l = xlo_d[:].rearrange("p (k c) -> p k c", k=KH)
    o3 = out_d[:].rearrange("p (k c) -> p k c", k=KH)

    with ExitStack() as ctx:
        tc = ctx.enter_context(tile.TileContext(nc))
        wpool = ctx.enter_context(tc.tile_pool(name="w", bufs=1))
        ipool = ctx.enter_context(tc.tile_pool(name="inter", bufs=1))
        spool = ctx.enter_context(tc.tile_pool(name="stage", bufs=1))
        ppool = ctx.enter_context(tc.tile_pool(name="psum", bufs=1, space="PSUM"))
        ppool2 = ctx.enter_context(tc.tile_pool(name="psum2", bufs=2, space="PSUM"))

        # weights resident: every load writes a fresh region
        ghi_sb = wpool.tile([P, KI, KH, P], f8, name="ghi_sb")
        glo_sb = wpool.tile([P, KI, KH, P], f8, name="glo_sb")
        uhi_sb = wpool.tile([P, KI, KH, P], f8, name="uhi_sb")
        ulo_sb = wpool.tile([P, KI, KH, P], f8, name="ulo_sb")
        dhi_sb = wpool.tile([P, KI2, H], f8, name="dhi_sb")
        dlo_sb = wpool.tile([P, KI2, H], f8, name="dlo_sb")
        sw_dmas = []

        def wblk(im):
            sw_dmas.append(nc.gpsimd.dma_start(ghi_sb[:, im], g3[:, im]))
            sw_dmas.append(nc.gpsimd.dma_start(glo_sb[:, im], gl3[:, im]))
            sw_dmas.append(nc.gpsimd.dma_start(uhi_sb[:, im], u3[:, im]))
            sw_dmas.append(nc.gpsimd.dma_start(ulo_sb[:, im], ul3[:, im]))

        # x pieces as separate write-once tensors (hazard tracking is
        # tensor-granular, so each matmul deps only its own piece's DMA);
        # out is a small ring that never aliases x, so out writes/stores
        # carry no x hazards at all
        xp = [
            wpool.tile([P, KH // 4, C], f8, name=f"xp{i}") for i in range(8)
        ]
        out_r = wpool.tile([P, KH // 4, C], bf16, name="out_r")

        # stream order is what the (serialized) DMA engine model follows:
        # first gate weights + the xh pieces that unlock P1/P3 work, then
        # up weights + the xl pieces, then the weight tail. x pieces ride
        # the otherwise idle ACT/DVE HWDGE rings.
        R = KH // 4  # x piece rows
        x_dmas = []

        def xblk(i):
            src = x3h if i < 4 else x3l
            j = i % 4
            x_dmas.append(
                nc.gpsimd.dma_start(xp[i][:], src[:, j * R : (j + 1) * R]))

        # tightest prologue: the first chain's operands arrive in exactly
        # the order its matmuls consume them
        sw_dmas.append(nc.gpsimd.dma_start(ghi_sb[:, 0], g3[:, 0]))
        xblk(0)
        sw_dmas.append(nc.gpsimd.dma_start(glo_sb[:, 0], gl3[:, 0]))
        xblk(1)
        sw_dmas.append(nc.gpsimd.dma_start(uhi_sb[:, 0], u3[:, 0]))
        xblk(2)
        sw_dmas.append(nc.gpsimd.dma_start(ulo_sb[:, 0], ul3[:, 0]))
        xblk(3)
        wblk(1)
        for i in range(4, 8):
            xblk(i)
        for im in range(2, KI):
            wblk(im)
        sw_dmas.append(nc.gpsimd.dma_start(
            dhi_sb[:], dhi_d[:].rearrange("p (i h) -> p i h", i=KI2)))
        sw_dmas.append(nc.gpsimd.dma_start(
            dlo_sb[:], dlo_d[:].rearrange("p (i h) -> p i h", i=KI2)))

        def xsl(plane, k, o, w):
            # x operand slice for k-subtile pair (k, k+1) of a plane
            t = xp[plane * 4 + k // R]
            kk = k % R
            return t[:, kk : kk + 2, o : o + w]

        ivl_sb = ipool.tile([P, KI, 2, C], f8, name="ivl_sb")
        # double-depth rotating staging (im%2, chunk): a slot's DVE readers
        # from round im-2 are absorbed into the PE proc by a per-im fence
        # that never stalls (two full rounds of PE work in between), so the
        # ACT rewrites and DVE ops carry no cross-WAR waits of their own
        t1_sb = spool.tile([P, 2, NJ, 512], bf16, name="t1_sb")
        v_sb = spool.tile([P, 2, NJ, 512], bf16, name="v_sb")
        # fence scratch: never written by real producers, so fence
        # reads/writes of their disjoint per-round columns carry no deps
        # beyond the explicit edges. Wait dedup is exact-instruction per
        # engine, so each fence lists the precise producer set (one
        # semaphore) it absorbs for its queue.
        fence_t = spool.tile([P, 64], bf16, name="fence_t")
        if _INIT_FENCE:
            nc.vector.memset(fence_t[:], 0.0)
        last_tt = [None]
        dve_by_im = {}
        act_by_im = {}

        # ---- phase 1: interT = silu(gateT) * upT, requantized hi/lo ----
        def chain24(psum, whi, wlo, im, o, w):
            # hi-plane products grouped by x piece (so each arriving piece
            # unlocks work during the prologue), lo-plane products last
            first = True
            for k in range(0, KH, 2):
                nc.tensor.matmul(
                    psum[:, :w], whi[:, im, k : k + 2], xsl(0, k, o, w),
                    start=first, stop=False, perf_mode=DR)
                first = False
                nc.tensor.matmul(
                    psum[:, :w], wlo[:, im, k : k + 2], xsl(0, k, o, w),
                    start=False, stop=False, perf_mode=DR)
            for k in range(0, KH, 2):
                nc.tensor.matmul(
                    psum[:, :w], whi[:, im, k : k + 2], xsl(1, k, o, w),
                    start=False, stop=(k == KH - 2), perf_mode=DR)

        for im in range(KI):
            s = im % 2
            if im >= 2:
                # absorb round im-2's exact producer sets into each queue:
                # af takes its DVE ops (one DVE sem), af2 its ACT ops (own
                # sem), df its DVE ops for the DVE queue. The staging
                # slots' WAR/WAW hazards this round then dedup away and
                # every real ACT/DVE instruction keeps a single wait.
                ca = 16 + 2 * (im - 2)
                af = nc.scalar.copy(
                    fence_t[:1, ca : ca + 2], fence_t[:1, ca : ca + 2])
                for bi in dve_by_im[im - 2]:
                    add_dep_helper(af.ins, bi.ins, sync=True,
                                   reason="act-dve slot fence")
                ca += 20
                af2 = nc.scalar.copy(
                    fence_t[:1, ca : ca + 2], fence_t[:1, ca : ca + 2])
                for bi in act_by_im[im - 2]:
                    add_dep_helper(af2.ins, bi.ins, sync=True,
                                   reason="act-act slot fence")
                # df's WAW against the previous df lands on the same DVE
                # semaphore as its edges, so one fixed column suffices
                df = nc.vector.tensor_copy(
                    fence_t[:1, 56:58], fence_t[:1, 56:58])
                for bi in dve_by_im[im - 2]:
                    add_dep_helper(df.ins, bi.ins, sync=True,
                                   reason="dve slot fence")
            pg = [
                ppool.tile([P, w], f32, tag=f"a{j}", name=f"pg{j}")
                for j, (_, w) in enumerate(chunks)
            ]
            pu = [
                ppool.tile([P, w], f32, tag=f"b{j}", name=f"pu{j}")
                for j, (_, w) in enumerate(chunks)
            ]
            for j, (o, w) in enumerate(chunks):
                chain24(pg[j], ghi_sb, glo_sb, im, o, w)
                chain24(pu[j], uhi_sb, ulo_sb, im, o, w)
            dve_by_im[im] = []
            act_by_im[im] = []
            for j, (o, w) in enumerate(chunks):
                # t1 = silu(g) (scale folds the fp8 scaling out before the
                # nonlinearity); v = u * S_I, then in-place v = inter * S_I
                a1 = nc.scalar.activation(
                    t1_sb[:, s, j, :w], pg[j][:],
                    mybir.ActivationFunctionType.Silu, scale=1.0 / (S_W * S_X))
                a2 = nc.scalar.activation(
                    v_sb[:, s, j, :w], pu[j][:],
                    mybir.ActivationFunctionType.Copy, scale=S_I / (S_W * S_X))
                # DVE chain: mul reads two ACT products (one collapsed sem),
                # copy/sub read only DVE-produced data (self sem)
                d1 = nc.vector.tensor_mul(
                    v_sb[:, s, j, :w], t1_sb[:, s, j, :w], v_sb[:, s, j, :w])
                d2 = nc.vector.tensor_copy(
                    ivl_sb[:, im, 0, o : o + w], v_sb[:, s, j, :w])
                d3 = nc.vector.tensor_sub(
                    ivl_sb[:, im, 1, o : o + w],
                    v_sb[:, s, j, :w],
                    ivl_sb[:, im, 0, o : o + w],
                )
                last_tt[0] = d3
                act_by_im[im] += [a1, a2]
                dve_by_im[im] += [d1, d2, d3]

        # ---- phase 2: outT = down_w @ interT ----
        # absorb every x-DMA completion into the ACT proc and the SP queue
        # (exact-instruction dedup: later out writes/stores on the aliased
        # slab then drop their WAW-vs-x deps and carry only PE/ACT). The
        # ACT fences use dep-free fence columns; the SP dummies read tiny
        # x slices whose RAW lands on the same DMA they absorb.

        # absorb the last TT's DVE tick into the PE proc so phase-2 matmuls
        # wait only on their dw DMA lane
        pe_fence = nc.tensor.ldweights(ghi_sb[:, 0, 0, 0:1])
        add_dep_helper(pe_fence.ins, last_tt[0].ins, sync=True, reason="pe fence")
        hw_dmas = list(x_dmas)
        tail_insts = []
        last_mm = []
        for hm in range(KH):
            po = [
                ppool2.tile([P, w], f32, tag=f"c{j}", name=f"po{j}")
                for j, (_, w) in enumerate(chunks)
            ]
            for j, (o, w) in enumerate(chunks):
                hs = hm * P
                n = 0
                for k in range(0, KI - 1, 2):
                    nc.tensor.matmul(
                        po[j][:, :w], dhi_sb[:, k : k + 2, hs : hs + P],
                        ivl_sb[:, k : k + 2, 0, o : o + w],
                        start=(n == 0), stop=False, perf_mode=DR)
                    n += 1
                for k in range(0, KI - 1, 2):
                    nc.tensor.matmul(
                        po[j][:, :w], dhi_sb[:, k : k + 2, hs : hs + P],
                        ivl_sb[:, k : k + 2, 1, o : o + w],
                        start=False, stop=False, perf_mode=DR)
                for k in range(0, KI - 1, 2):
                    nc.tensor.matmul(
                        po[j][:, :w], dlo_sb[:, k : k + 2, hs : hs + P],
                        ivl_sb[:, k : k + 2, 0, o : o + w],
                        start=False, stop=False, perf_mode=DR)
                # odd last subtile: its hi/lo planes are adjacent in ivl,
                # paired against the duplicated stationary slot KI..KI+1
                nc.tensor.matmul(
                    po[j][:, :w], dhi_sb[:, KI - 1 : KI + 1, hs : hs + P],
                    ivl_sb[:, KI - 1, 0:2, o : o + w],
                    start=False, stop=False, perf_mode=DR)
                last_mm.append(nc.tensor.matmul(
                    po[j][:, :w], dlo_sb[:, KI - 1 : KI + 1, hs : hs + P],
                    ivl_sb[:, KI - 1, 0:2, o : o + w],
                    start=False, stop=True, perf_mode=DR))
                del last_mm[:-1]
            q, qr = hm // (KH // 4), hm % (KH // 4)
            if qr == 0 and q >= 1:
                # ring handoff: absorb the previous quarter's copies (ACT
                # self sem) and its two pair stores (one fence per DMA
                # semaphore) so this quarter's copies keep their single
                # PE wait
                f3a = nc.scalar.copy(fence_t[:1, 0:2], fence_t[:1, 0:2])
                for bi in tail_insts[-2 * (KH // 4):]:
                    add_dep_helper(f3a.ins, bi.ins, sync=True,
                                   reason="ring copy fence")
                ca = 2 + 2 * (q - 1)
                f3b = nc.scalar.copy(
                    fence_t[:1, ca : ca + 2], fence_t[:1, ca : ca + 2])
                add_dep_helper(f3b.ins, hw_dmas[-1].ins, sync=True,
                               reason="ring store fence")
            for j, (o, w) in enumerate(chunks):
                tail_insts.append(nc.scalar.activation(
                    out_r[:, qr, o : o + w], po[j][:],
                    mybir.ActivationFunctionType.Copy, scale=1.0 / (S_W * S_I)))
            # quarter stores for q0-q2; the last quarter tapers (pair,
            # single, single) so the tail store after the final copy is
            # one row. 6 SP DMAs total, each on a fresh HW lane.
            if q < 3:
                if qr == KH // 4 - 1:
                    hw_dmas.append(nc.sync.dma_start(
                        o3[:, hm - qr : hm + 1, :], out_r[:]
                    ))
            elif qr == 1:
                hw_dmas.append(nc.sync.dma_start(
                    o3[:, hm - 1 : hm + 1, :], out_r[:, 0:2]
                ))
            elif qr >= 2:
                hw_dmas.append(nc.sync.dma_start(
                    o3[:, hm : hm + 1, :], out_r[:, qr : qr + 1]
                ))

        # pre-drain: absorb every proc's final tick into the SP sequencer
        # one sync edge at a time, so the kernel-tail drain's waits (which
        # would exceed the instruction's wait slots) are all elided
        for insts in (sw_dmas, hw_dmas, [last_tt[0]], tail_insts[-2:], last_mm[-1:]):
            for bi in insts:
                if bi is None:
                    continue
                nop = nc.sync.nop()
                add_dep_helper(nop.ins, bi.ins, sync=True, reason="pre-drain")

    return nc


def _split_fp8(a, scale):
    """Split scale*a into hi/lo float8_e4m3 planes (as fp8 arrays)."""
    import ml_dtypes

    f8 = ml_dtypes.float8_e4m3
    v = (a * scale).astype(np.float32)
    hi = v.astype(f8)
    lo = (v - hi.astype(np.float32)).astype(f8)
    return hi, lo


def kernel(hidden_states, top_k_index, top_k_weights, gate_w, up_w, down_w):
    import ml_dtypes
    from concourse.bass_utils import run_bass_kernel_spmd

    hs = np.ascontiguousarray(np.asarray(hidden_states, dtype=np.float32))
    tki = np.asarray(top_k_index)
    tkw = np.asarray(top_k_weights, dtype=np.float32)
    gw = np.asarray(gate_w, dtype=np.float32)
    uw = np.asarray(up_w, dtype=np.float32)
    dw = np.asarray(down_w, dtype=np.float32)

    T, H = hs.shape
    E, I, _ = gw.shape
    P = 128
    KH = H // P
    KI = I // P
    KI2 = KI + 1

    tok_lists, w_lists = [], []
    for e in range(E):
        mask = tki == e
        toks = np.nonzero(mask.any(axis=1))[0]
        w = (tkw * mask).sum(axis=1)[toks].astype(np.float32)
        tok_lists.append(toks)
        w_lists.append(w)

    # capacity: smallest multiple of 16 covering the busiest expert
    # (1008 for the balanced T=4096, K=2, E=8 regime); experts with more
    # tokens spill into additional SPMD rounds
    maxn = max(len(t) for t in tok_lists)
    C = min(1024, -(-maxn // 16) * 16)
    n_rounds = max(1, -(-maxn // C))

    def pack_gu(a):  # [I, H] fp8 -> [P, KI*KH*P] im-block-major
        # column block im of a.T[H, I] as [P, KH, 128], contiguous per block
        b = a.T.reshape(KH, P, KI, P).transpose(1, 2, 0, 3)  # p, im, kh, q
        return np.ascontiguousarray(b.reshape(P, KI * KH * P))

    def pack_d(a):  # [H, I] fp8 -> [P, KI2*H] with duplicated last subtile
        b = a.T.reshape(KI, P, H).transpose(1, 0, 2)  # p, ki, h
        b = np.concatenate([b, b[:, KI - 1 : KI]], axis=1)  # dup slot
        return np.ascontiguousarray(b.reshape(P, KI2 * H))

    wmaps = []
    for e in range(E):
        ghi, glo = _split_fp8(gw[e], S_W)
        uhi, ulo = _split_fp8(uw[e], S_W)
        dhi, dlo = _split_fp8(dw[e], S_W)
        wmaps.append({
            "ghi": pack_gu(ghi), "glo": pack_gu(glo),
            "uhi": pack_gu(uhi), "ulo": pack_gu(ulo),
            "dhi": pack_d(dhi), "dlo": pack_d(dlo),
        })

    nc = _build_bass(C, H, I)
    out = np.zeros((T, H), np.float32)
    global _last_results
    for r in range(n_rounds):
        in_maps = []
        for e in range(E):
            toks = tok_lists[e][r * C : (r + 1) * C]
            xT = np.zeros((H, C), np.float32)
            xT[:, : len(toks)] = hs[toks].T
            xhi, xlo = _split_fp8(xT, S_X)

            def pack_x(a):  # [H, C] -> [P, KH*C]
                b = a.reshape(KH, P, C).transpose(1, 0, 2)
                return np.ascontiguousarray(b.reshape(P, KH * C))

            in_maps.append({"xhi": pack_x(xhi), "xlo": pack_x(xlo), **wmaps[e]})
        res = run_bass_kernel_spmd(nc, in_maps, core_ids=list(range(E)))
        _last_results = res
        for e in range(E):
            toks = tok_lists[e][r * C : (r + 1) * C]
            n = len(toks)
            if n == 0:
                continue
            o = np.asarray(res.results[e]["out"]).astype(np.float32)  # [P, KH*C]
            outT = o.reshape(P, KH, C).transpose(1, 0, 2).reshape(H, C)
            out[toks] += w_lists[e][r * C : r * C + n, None] * outT[:, :n].T
    return out


# revision 67
# speedup vs baseline: 1.4707x; 1.0077x over previous
"""MoE SwiGLU experts kernel for Trainium2 (8 NeuronCores, expert-parallel).

Each core owns one expert e. Host does the dispatch (gathers tokens whose
top-k includes e, dedups with summed combine weights), splits every matmul
operand into two fp8-e4m3 planes (hi = fp8(v), lo = fp8(v - hi), with
power-of-two scales so the uniform-[-1/sqrt(H)] weights clear the fp8
denormal floor), and pre-swizzles everything to partition-major layouts.
Device computes the SwiGLU MLP with fp8 DoubleRow matmuls: each
instruction contracts two 128-row k-subtiles, and the three significant
cross-products (hi*hi, hi*lo, lo*hi) recover ~bf16 accuracy at half the
PE cycles of bf16:

    gateT = gate_w[e] @ x_eT          # [I, C]  (contract H, 24 instrs/chain)
    upT   = up_w[e]   @ x_eT          # [I, C]
    interT = silu(gateT) * upT        # [I, C]  requantized to fp8 hi/lo
    outT  = down_w[e]  @ interT       # [H, C]  (contract I, 17 instrs/chain)

The intermediate is stored hi/lo-interleaved [P, KI, 2, C] so phase-2
moving operands pair adjacent k-subtiles (planes strided) and the odd
11th subtile pairs its own hi/lo planes against a duplicated stationary
slot. Host scatter-adds w * outT.T rows into the [T, H] output.

Hardware sync-wait slots per instruction are scarce (walrus rejects
kernels that need too many), so the structure keeps every instruction's
dependency fan-in tiny:
- all weight planes and the token slab are fully SBUF-resident and
  written once, so their DMAs carry no WAR/WAW waits;
- the evac pipeline alternates ACT/DVE so same-engine program order
  subsumes most deps; slot-rotating staging is rewritten only after its
  single cross-engine reader ran;
- output stores are batched so each lands on a fresh HW DMA lane.
"""

import numpy as np

# power-of-two scales: keep fp8 operands out of the denormal floor and
# under the e4m3 max (240); all folded back via ACT scale params
S_W = float(2.0**12)
S_X = float(2.0**4)
S_I = float(2.0**3)

# CoreSim-only: initialize the fence scratch so the interpreter's
# uninitialized-read check passes. The device build must NOT set this -
# fence reads of unwritten scratch are intentionally dependency-free.
_INIT_FENCE = False


def _build_bass(C: int, H: int, I: int):
    from contextlib import ExitStack

    import concourse.bass as bass
    import concourse.mybir as mybir
    import concourse.tile as tile
    from concourse.tile import add_dep_helper

    f32 = mybir.dt.float32
    bf16 = mybir.dt.bfloat16
    f8 = mybir.dt.float8e4
    P = 128
    KH = H // P  # 16
    KI = I // P  # 11
    KI2 = KI + 1  # down stationary gets a duplicate of the odd last subtile
    DR = mybir.MatmulPerfMode.DoubleRow

    chunks = []
    off = 0
    while off < C:
        w = min(512, C - off)
        chunks.append((off, w))
        off += w
    NJ = len(chunks)

    nc = bass.Bass(dynamic_dma_scratch_size=8192)
    # all inputs pre-swizzled on host to [P, ...] partition-major layouts
    xhi_d = nc.dram_tensor("xhi", [P, KH * C], f8, kind="ExternalInput")
    xlo_d = nc.dram_tensor("xlo", [P, KH * C], f8, kind="ExternalInput")
    ghi_d = nc.dram_tensor("ghi", [P, KI * KH * P], f8, kind="ExternalInput")
    glo_d = nc.dram_tensor("glo", [P, KI * KH * P], f8, kind="ExternalInput")
    uhi_d = nc.dram_tensor("uhi", [P, KI * KH * P], f8, kind="ExternalInput")
    ulo_d = nc.dram_tensor("ulo", [P, KI * KH * P], f8, kind="ExternalInput")
    dhi_d = nc.dram_tensor("dhi", [P, KI2 * H], f8, kind="ExternalInput")
    dlo_d = nc.dram_tensor("dlo", [P, KI2 * H], f8, kind="ExternalInput")
    out_d = nc.dram_tensor("out", [P, KH * C], bf16, kind="ExternalOutput")

    g3 = ghi_d[:].rearrange("p (i k q) -> p i k q", i=KI, k=KH)
    gl3 = glo_d[:].rearrange("p (i k q) -> p i k q", i=KI, k=KH)
    u3 = uhi_d[:].rearrange("p (i k q) -> p i k q", i=KI, k=KH)
    ul3 = ulo_d[:].rearrange("p (i k q) -> p i k q", i=KI, k=KH)
    x3h = xhi_d[:].rearrange("p (k c) -> p k c", k=KH)
    x3l = xlo_d[:].rearrange("p (k c) -> p k c", k=KH)
    o3 = out_d[:].rearrange("p (k c) -> p k c", k=KH)

    with ExitStack() as ctx:
        tc = ctx.enter_context(tile.TileContext(nc))
        wpool = ctx.enter_context(tc.tile_pool(name="w", bufs=1))
        ipool = ctx.enter_context(tc.tile_pool(name="inter", bufs=1))
        spool = ctx.enter_context(tc.tile_pool(name="stage", bufs=1))
        ppool = ctx.enter_context(tc.tile_pool(name="psum", bufs=1, space="PSUM"))
        ppool2 = ctx.enter_context(tc.tile_pool(name="psum2", bufs=2, space="PSUM"))

        # weights resident: every load writes a fresh region
        ghi_sb = wpool.tile([P, KI, KH, P], f8, name="ghi_sb")
        glo_sb = wpool.tile([P, KI, KH, P], f8, name="glo_sb")
        uhi_sb = wpool.tile([P, KI, KH, P], f8, name="uhi_sb")
        ulo_sb = wpool.tile([P, KI, KH, P], f8, name="ulo_sb")
        dhi_sb = wpool.tile([P, KI2, H], f8, name="dhi_sb")
        dlo_sb = wpool.tile([P, KI2, H], f8, name="dlo_sb")
        sw_dmas = []

        def wblk(im):
            sw_dmas.append(nc.gpsimd.dma_start(ghi_sb[:, im], g3[:, im]))
            sw_dmas.append(nc.gpsimd.dma_start(glo_sb[:, im], gl3[:, im]))
            sw_dmas.append(nc.gpsimd.dma_start(uhi_sb[:, im], u3[:, im]))
            sw_dmas.append(nc.gpsimd.dma_start(ulo_sb[:, im], ul3[:, im]))

        # x pieces as separate write-once tensors (hazard tracking is
        # tensor-granular, so each matmul deps only its own piece's DMA);
        # out is a small ring that never aliases x, so out writes/stores
        # carry no x hazards at all
        xp = [
            wpool.tile([P, KH // 4, C], f8, name=f"xp{i}") for i in range(8)
        ]
        out_r = wpool.tile([P, KH // 4, C], bf16, name="out_r")

        # stream order is what the (serialized) DMA engine model follows:
        # first gate weights + the xh pieces that unlock P1/P3 work, then
        # up weights + the xl pieces, then the weight tail. x pieces ride
        # the otherwise idle ACT/DVE HWDGE rings.
        R = KH // 4  # x piece rows
        x_dmas = []

        def xblk(i):
            src = x3h if i < 4 else x3l
            j = i % 4
            x_dmas.append(
                nc.gpsimd.dma_start(xp[i][:], src[:, j * R : (j + 1) * R]))

        # tightest prologue: the first chain's operands arrive in exactly
        # the order its matmuls consume them; the very first two ride the
        # SP HWDGE ring, which starts ~1.4us sooner than the SWDGE ring
        sw_dmas.append(nc.sync.dma_start(ghi_sb[:, 0], g3[:, 0]))
        x_dmas.append(nc.sync.dma_start(xp[0][:], x3h[:, 0:R]))
        sw_dmas.append(nc.gpsimd.dma_start(glo_sb[:, 0], gl3[:, 0]))
        xblk(1)
        sw_dmas.append(nc.gpsimd.dma_start(uhi_sb[:, 0], u3[:, 0]))
        xblk(2)
        sw_dmas.append(nc.gpsimd.dma_start(ulo_sb[:, 0], ul3[:, 0]))
        xblk(3)
        # interleave the second weight block with the xl pieces: each
        # stream stays just ahead of the matmuls that consume it
        sw_dmas.append(nc.gpsimd.dma_start(ghi_sb[:, 1], g3[:, 1]))
        xblk(4)
        sw_dmas.append(nc.gpsimd.dma_start(glo_sb[:, 1], gl3[:, 1]))
        xblk(5)
        sw_dmas.append(nc.gpsimd.dma_start(uhi_sb[:, 1], u3[:, 1]))
        xblk(6)
        sw_dmas.append(nc.gpsimd.dma_start(ulo_sb[:, 1], ul3[:, 1]))
        xblk(7)
        for im in range(2, KI):
            wblk(im)
        sw_dmas.append(nc.gpsimd.dma_start(
            dhi_sb[:], dhi_d[:].rearrange("p (i h) -> p i h", i=KI2)))
        sw_dmas.append(nc.gpsimd.dma_start(
            dlo_sb[:], dlo_d[:].rearrange("p (i h) -> p i h", i=KI2)))

        def xsl(plane, k, o, w):
            # x operand slice for k-subtile pair (k, k+1) of a plane
            t = xp[plane * 4 + k // R]
            kk = k % R
            return t[:, kk : kk + 2, o : o + w]

        ivl_sb = ipool.tile([P, KI, 2, C], f8, name="ivl_sb")
        # double-depth rotating staging (im%2, chunk): a slot's DVE readers
        # from round im-2 are absorbed into the PE proc by a per-im fence
        # that never stalls (two full rounds of PE work in between), so the
        # ACT rewrites and DVE ops carry no cross-WAR waits of their own
        t1_sb = spool.tile([P, 2, NJ, 512], bf16, name="t1_sb")
        v_sb = spool.tile([P, 2, NJ, 512], bf16, name="v_sb")
        # fence scratch: never written by real producers, so fence
        # reads/writes of their disjoint per-round columns carry no deps
        # beyond the explicit edges. Wait dedup is exact-instruction per
        # engine, so each fence lists the precise producer set (one
        # semaphore) it absorbs for its queue.
        fence_t = spool.tile([P, 64], bf16, name="fence_t")
        if _INIT_FENCE:
            nc.vector.memset(fence_t[:], 0.0)
        last_tt = [None]
        dve_by_im = {}
        act_by_im = {}

        # ---- phase 1: interT = silu(gateT) * upT, requantized hi/lo ----
        def chain24(psum, whi, wlo, im, o, w):
            # hi-plane products grouped by x piece (so each arriving piece
            # unlocks work during the prologue), lo-plane products last
            first = True
            for k in range(0, KH, 2):
                nc.tensor.matmul(
                    psum[:, :w], whi[:, im, k : k + 2], xsl(0, k, o, w),
                    start=first, stop=False, perf_mode=DR)
                first = False
                nc.tensor.matmul(
                    psum[:, :w], wlo[:, im, k : k + 2], xsl(0, k, o, w),
                    start=False, stop=False, perf_mode=DR)
            for k in range(0, KH, 2):
                nc.tensor.matmul(
                    psum[:, :w], whi[:, im, k : k + 2], xsl(1, k, o, w),
                    start=False, stop=(k == KH - 2), perf_mode=DR)

        for im in range(KI):
            s = im % 2
            if im >= 2:
                # absorb round im-2's exact producer sets into each queue:
                # af takes its DVE ops (one DVE sem), af2 its ACT ops (own
                # sem), df its DVE ops for the DVE queue. The staging
                # slots' WAR/WAW hazards this round then dedup away and
                # every real ACT/DVE instruction keeps a single wait.
                ca = 16 + 2 * (im - 2)
                af = nc.scalar.copy(
                    fence_t[:1, ca : ca + 2], fence_t[:1, ca : ca + 2])
                for bi in dve_by_im[im - 2]:
                    add_dep_helper(af.ins, bi.ins, sync=True,
                                   reason="act-dve slot fence")
                ca += 20
                af2 = nc.scalar.copy(
                    fence_t[:1, ca : ca + 2], fence_t[:1, ca : ca + 2])
                for bi in act_by_im[im - 2]:
                    add_dep_helper(af2.ins, bi.ins, sync=True,
                                   reason="act-act slot fence")
                # df's WAW against the previous df lands on the same DVE
                # semaphore as its edges, so one fixed column suffices
                df = nc.vector.tensor_copy(
                    fence_t[:1, 56:58], fence_t[:1, 56:58])
                for bi in dve_by_im[im - 2]:
                    add_dep_helper(df.ins, bi.ins, sync=True,
                                   reason="dve slot fence")
            pg = [
                ppool.tile([P, w], f32, tag=f"a{j}", name=f"pg{j}")
                for j, (_, w) in enumerate(chunks)
            ]
            pu = [
                ppool.tile([P, w], f32, tag=f"b{j}", name=f"pu{j}")
                for j, (_, w) in enumerate(chunks)
            ]
            for j, (o, w) in enumerate(chunks):
                chain24(pg[j], ghi_sb, glo_sb, im, o, w)
                chain24(pu[j], uhi_sb, ulo_sb, im, o, w)
            dve_by_im[im] = []
            act_by_im[im] = []
            for j, (o, w) in enumerate(chunks):
                # t1 = silu(g) (scale folds the fp8 scaling out before the
                # nonlinearity); v = u * S_I, then in-place v = inter * S_I
                a1 = nc.scalar.activation(
                    t1_sb[:, s, j, :w], pg[j][:],
                    mybir.ActivationFunctionType.Silu, scale=1.0 / (S_W * S_X))
                a2 = nc.scalar.activation(
                    v_sb[:, s, j, :w], pu[j][:],
                    mybir.ActivationFunctionType.Copy, scale=S_I / (S_W * S_X))
                # DVE chain: mul reads two ACT products (one collapsed sem),
                # copy/sub read only DVE-produced data (self sem)
                d1 = nc.vector.tensor_mul(
                    v_sb[:, s, j, :w], t1_sb[:, s, j, :w], v_sb[:, s, j, :w])
                d2 = nc.vector.tensor_copy(
                    ivl_sb[:, im, 0, o : o + w], v_sb[:, s, j, :w])
                d3 = nc.vector.tensor_sub(
                    ivl_sb[:, im, 1, o : o + w],
                    v_sb[:, s, j, :w],
                    ivl_sb[:, im, 0, o : o + w],
                )
                last_tt[0] = d3
                act_by_im[im] += [a1, a2]
                dve_by_im[im] += [d1, d2, d3]

        # ---- phase 2: outT = down_w @ interT ----
        # absorb every x-DMA completion into the ACT proc and the SP queue
        # (exact-instruction dedup: later out writes/stores on the aliased
        # slab then drop their WAW-vs-x deps and carry only PE/ACT). The
        # ACT fences use dep-free fence columns; the SP dummies read tiny
        # x slices whose RAW lands on the same DMA they absorb.

        # absorb the last TT's DVE tick into the PE proc so phase-2 matmuls
        # wait only on their dw DMA lane
        pe_fence = nc.tensor.ldweights(ghi_sb[:, 0, 0, 0:1])
        add_dep_helper(pe_fence.ins, last_tt[0].ins, sync=True, reason="pe fence")
        hw_dmas = list(x_dmas)
        tail_insts = []
        last_mm = []
        for hm in range(KH):
            po = [
                ppool2.tile([P, w], f32, tag=f"c{j}", name=f"po{j}")
                for j, (_, w) in enumerate(chunks)
            ]
            for j, (o, w) in enumerate(chunks):
                hs = hm * P
                n = 0
                for k in range(0, KI - 1, 2):
                    nc.tensor.matmul(
                        po[j][:, :w], dhi_sb[:, k : k + 2, hs : hs + P],
                        ivl_sb[:, k : k + 2, 0, o : o + w],
                        start=(n == 0), stop=False, perf_mode=DR)
                    n += 1
                for k in range(0, KI - 1, 2):
                    nc.tensor.matmul(
                        po[j][:, :w], dhi_sb[:, k : k + 2, hs : hs + P],
                        ivl_sb[:, k : k + 2, 1, o : o + w],
                        start=False, stop=False, perf_mode=DR)
                for k in range(0, KI - 1, 2):
                    nc.tensor.matmul(
                        po[j][:, :w], dlo_sb[:, k : k + 2, hs : hs + P],
                        ivl_sb[:, k : k + 2, 0, o : o + w],
                        start=False, stop=False, perf_mode=DR)
                # odd last subtile: its hi/lo planes are adjacent in ivl,
                # paired against the duplicated stationary slot KI..KI+1
                nc.tensor.matmul(
                    po[j][:, :w], dhi_sb[:, KI - 1 : KI + 1, hs : hs + P],
                    ivl_sb[:, KI - 1, 0:2, o : o + w],
                    start=False, stop=False, perf_mode=DR)
                last_mm.append(nc.tensor.matmul(
                    po[j][:, :w], dlo_sb[:, KI - 1 : KI + 1, hs : hs + P],
                    ivl_sb[:, KI - 1, 0:2, o : o + w],
                    start=False, stop=True, perf_mode=DR))
                del last_mm[:-1]
            q, qr = hm // (KH // 4), hm % (KH // 4)
            if qr == 0 and q >= 1:
                # ring handoff: absorb the previous quarter's copies (ACT
                # self sem) and its two pair stores (one fence per DMA
                # semaphore) so this quarter's copies keep their single
                # PE wait
                f3a = nc.scalar.copy(fence_t[:1, 0:2], fence_t[:1, 0:2])
                for bi in tail_insts[-2 * (KH // 4):]:
                    add_dep_helper(f3a.ins, bi.ins, sync=True,
                                   reason="ring copy fence")
                ca = 2 + 2 * (q - 1)
                f3b = nc.scalar.copy(
                    fence_t[:1, ca : ca + 2], fence_t[:1, ca : ca + 2])
                add_dep_helper(f3b.ins, hw_dmas[-1].ins, sync=True,
                               reason="ring store fence")
            for j, (o, w) in enumerate(chunks):
                tail_insts.append(nc.scalar.activation(
                    out_r[:, qr, o : o + w], po[j][:],
                    mybir.ActivationFunctionType.Copy, scale=1.0 / (S_W * S_I)))
            # quarter stores for q0-q2; the last quarter tapers (pair,
            # single, single) so the tail store after the final copy is
            # one row. 6 SP DMAs total, each on a fresh HW lane.
            if q < 3:
                if qr == KH // 4 - 1:
                    hw_dmas.append(nc.sync.dma_start(
                        o3[:, hm - qr : hm + 1, :], out_r[:]
                    ))
            elif qr == 1:
                hw_dmas.append(nc.sync.dma_start(
                    o3[:, hm - 1 : hm + 1, :], out_r[:, 0:2]
                ))
            elif qr >= 2:
                hw_dmas.append(nc.sync.dma_start(
                    o3[:, hm : hm + 1, :], out_r[:, qr : qr + 1]
                ))

        # pre-drain: absorb every proc's final tick into the SP sequencer
        # one sync edge at a time, so the kernel-tail drain's waits (which
        # would exceed the instruction's wait slots) are all elided
        for insts in (sw_dmas, hw_dmas, [last_tt[0]], tail_insts[-2:], last_mm[-1:]):
            for bi in insts:
                if bi is None:
                    continue
                nop = nc.sync.nop()
                add_dep_helper(nop.ins, bi.ins, sync=True, reason="pre-drain")

    return nc


def _split_fp8(a, scale):
    """Split scale*a into hi/lo float8_e4m3 planes (as fp8 arrays)."""
    import ml_dtypes

    f8 = ml_dtypes.float8_e4m3
    v = (a * scale).astype(np.float32)
    hi = v.astype(f8)
    lo = (v - hi.astype(np.float32)).astype(f8)
    return hi, lo


def kernel(hidden_states, top_k_index, top_k_weights, gate_w, up_w, down_w):
    import ml_dtypes
    from concourse.bass_utils import run_bass_kernel_spmd

    hs = np.ascontiguousarray(np.asarray(hidden_states, dtype=np.float32))
    tki = np.asarray(top_k_index)
    tkw = np.asarray(top_k_weights, dtype=np.float32)
    gw = np.asarray(gate_w, dtype=np.float32)
    uw = np.asarray(up_w, dtype=np.float32)
    dw = np.asarray(down_w, dtype=np.float32)

    T, H = hs.shape
    E, I, _ = gw.shape
    P = 128
    KH = H // P
    KI = I // P
    KI2 = KI + 1

    tok_lists, w_lists = [], []
    for e in range(E):
        mask = tki == e
        toks = np.nonzero(mask.any(axis=1))[0]
        w = (tkw * mask).sum(axis=1)[toks].astype(np.float32)
        tok_lists.append(toks)
        w_lists.append(w)

    # capacity: smallest multiple of 16 covering the busiest expert
    # (1008 for the balanced T=4096, K=2, E=8 regime); experts with more
    # tokens spill into additional SPMD rounds
    maxn = max(len(t) for t in tok_lists)
    C = min(1024, -(-maxn // 16) * 16)
    n_rounds = max(1, -(-maxn // C))

    def pack_gu(a):  # [I, H] fp8 -> [P, KI*KH*P] im-block-major
        # column block im of a.T[H, I] as [P, KH, 128], contiguous per block
        b = a.T.reshape(KH, P, KI, P).transpose(1, 2, 0, 3)  # p, im, kh, q
        return np.ascontiguousarray(b.reshape(P, KI * KH * P))

    def pack_d(a):  # [H, I] fp8 -> [P, KI2*H] with duplicated last subtile
        b = a.T.reshape(KI, P, H).transpose(1, 0, 2)  # p, ki, h
        b = np.concatenate([b, b[:, KI - 1 : KI]], axis=1)  # dup slot
        return np.ascontiguousarray(b.reshape(P, KI2 * H))

    wmaps = []
    for e in range(E):
        ghi, glo = _split_fp8(gw[e], S_W)
        uhi, ulo = _split_fp8(uw[e], S_W)
        dhi, dlo = _split_fp8(dw[e], S_W)
        wmaps.append({
            "ghi": pack_gu(ghi), "glo": pack_gu(glo),
            "uhi": pack_gu(uhi), "ulo": pack_gu(ulo),
            "dhi": pack_d(dhi), "dlo": pack_d(dlo),
        })

    nc = _build_bass(C, H, I)
    out = np.zeros((T, H), np.float32)
    global _last_results
    for r in range(n_rounds):
        in_maps = []
        for e in range(E):
            toks = tok_lists[e][r * C : (r + 1) * C]
            xT = np.zeros((H, C), np.float32)
            xT[:, : len(toks)] = hs[toks].T
            xhi, xlo = _split_fp8(xT, S_X)

            def pack_x(a):  # [H, C] -> [P, KH*C]
                b = a.reshape(KH, P, C).transpose(1, 0, 2)
                return np.ascontiguousarray(b.reshape(P, KH * C))

            in_maps.append({"xhi": pack_x(xhi), "xlo": pack_x(xlo), **wmaps[e]})
        res = run_bass_kernel_spmd(nc, in_maps, core_ids=list(range(E)))
        _last_results = res
        for e in range(E):
            toks = tok_lists[e][r * C : (r + 1) * C]
            n = len(toks)
            if n == 0:
                continue
            o = np.asarray(res.results[e]["out"]).astype(np.float32)  # [P, KH*C]
            outT = o.reshape(P, KH, C).transpose(1, 0, 2).reshape(H, C)
            out[toks] += w_lists[e][r * C : r * C + n, None] * outT[:, :n].T
    return out


# revision 74
# speedup vs baseline: 1.4766x; 1.0040x over previous
"""MoE SwiGLU experts kernel for Trainium2 (8 NeuronCores, expert-parallel).

Each core owns one expert e. Host does the dispatch (gathers tokens whose
top-k includes e, dedups with summed combine weights), splits every matmul
operand into two fp8-e4m3 planes (hi = fp8(v), lo = fp8(v - hi), with
power-of-two scales so the uniform-[-1/sqrt(H)] weights clear the fp8
denormal floor), and pre-swizzles everything to partition-major layouts.
Device computes the SwiGLU MLP with fp8 DoubleRow matmuls: each
instruction contracts two 128-row k-subtiles, and the three significant
cross-products (hi*hi, hi*lo, lo*hi) recover ~bf16 accuracy at half the
PE cycles of bf16:

    gateT = gate_w[e] @ x_eT          # [I, C]  (contract H, 24 instrs/chain)
    upT   = up_w[e]   @ x_eT          # [I, C]
    interT = silu(gateT) * upT        # [I, C]  requantized to fp8 hi/lo
    outT  = down_w[e]  @ interT       # [H, C]  (contract I, 17 instrs/chain)

The intermediate is stored hi/lo-interleaved [P, KI, 2, C] so phase-2
moving operands pair adjacent k-subtiles (planes strided) and the odd
11th subtile pairs its own hi/lo planes against a duplicated stationary
slot. Host scatter-adds w * outT.T rows into the [T, H] output.

Hardware sync-wait slots per instruction are scarce (walrus rejects
kernels that need too many), so the structure keeps every instruction's
dependency fan-in tiny:
- all weight planes and the token slab are fully SBUF-resident and
  written once, so their DMAs carry no WAR/WAW waits;
- the evac pipeline alternates ACT/DVE so same-engine program order
  subsumes most deps; slot-rotating staging is rewritten only after its
  single cross-engine reader ran;
- output stores are batched so each lands on a fresh HW DMA lane.
"""

import numpy as np

# power-of-two scales: keep fp8 operands out of the denormal floor and
# under the e4m3 max (240); all folded back via ACT scale params
S_W = float(2.0**12)
S_X = float(2.0**4)
S_I = float(2.0**3)

# CoreSim-only: initialize the fence scratch so the interpreter's
# uninitialized-read check passes. The device build must NOT set this -
# fence reads of unwritten scratch are intentionally dependency-free.
_INIT_FENCE = False


def _build_bass(C: int, H: int, I: int):
    from contextlib import ExitStack

    import concourse.bass as bass
    import concourse.mybir as mybir
    import concourse.tile as tile
    from concourse.tile import add_dep_helper

    f32 = mybir.dt.float32
    bf16 = mybir.dt.bfloat16
    f8 = mybir.dt.float8e4
    P = 128
    KH = H // P  # 16
    KI = I // P  # 11
    KI2 = KI + 1  # down stationary gets a duplicate of the odd last subtile
    DR = mybir.MatmulPerfMode.DoubleRow

    chunks = []
    off = 0
    while off < C:
        w = min(512, C - off)
        chunks.append((off, w))
        off += w
    NJ = len(chunks)

    nc = bass.Bass(dynamic_dma_scratch_size=8192)
    # all inputs pre-swizzled on host to [P, ...] partition-major layouts
    xhi_d = nc.dram_tensor("xhi", [P, KH * C], f8, kind="ExternalInput")
    xlo_d = nc.dram_tensor("xlo", [P, KH * C], f8, kind="ExternalInput")
    ghi_d = nc.dram_tensor("ghi", [P, KI * KH * P], f8, kind="ExternalInput")
    glo_d = nc.dram_tensor("glo", [P, KI * KH * P], f8, kind="ExternalInput")
    uhi_d = nc.dram_tensor("uhi", [P, KI * KH * P], f8, kind="ExternalInput")
    ulo_d = nc.dram_tensor("ulo", [P, KI * KH * P], f8, kind="ExternalInput")
    dhi_d = nc.dram_tensor("dhi", [P, KI2 * H], f8, kind="ExternalInput")
    dlo_d = nc.dram_tensor("dlo", [P, KI2 * H], f8, kind="ExternalInput")
    out_d = nc.dram_tensor("out", [P, KH * C], bf16, kind="ExternalOutput")

    g3 = ghi_d[:].rearrange("p (i k q) -> p i k q", i=KI, k=KH)
    gl3 = glo_d[:].rearrange("p (i k q) -> p i k q", i=KI, k=KH)
    u3 = uhi_d[:].rearrange("p (i k q) -> p i k q", i=KI, k=KH)
    ul3 = ulo_d[:].rearrange("p (i k q) -> p i k q", i=KI, k=KH)
    x3h = xhi_d[:].rearrange("p (k c) -> p k c", k=KH)
    x3l = xlo_d[:].rearrange("p (k c) -> p k c", k=KH)
    o3 = out_d[:].rearrange("p (k c) -> p k c", k=KH)

    with ExitStack() as ctx:
        tc = ctx.enter_context(tile.TileContext(nc))
        wpool = ctx.enter_context(tc.tile_pool(name="w", bufs=1))
        ipool = ctx.enter_context(tc.tile_pool(name="inter", bufs=1))
        spool = ctx.enter_context(tc.tile_pool(name="stage", bufs=1))
        ppool = ctx.enter_context(tc.tile_pool(name="psum", bufs=1, space="PSUM"))
        ppool2 = ctx.enter_context(tc.tile_pool(name="psum2", bufs=2, space="PSUM"))

        # weights resident: every load writes a fresh region
        ghi_sb = wpool.tile([P, KI, KH, P], f8, name="ghi_sb")
        glo_sb = wpool.tile([P, KI, KH, P], f8, name="glo_sb")
        uhi_sb = wpool.tile([P, KI, KH, P], f8, name="uhi_sb")
        ulo_sb = wpool.tile([P, KI, KH, P], f8, name="ulo_sb")
        dhi_sb = wpool.tile([P, KI2, H], f8, name="dhi_sb")
        dlo_sb = wpool.tile([P, KI2, H], f8, name="dlo_sb")
        sw_dmas = []

        def wblk(im):
            sw_dmas.append(nc.gpsimd.dma_start(ghi_sb[:, im], g3[:, im]))
            sw_dmas.append(nc.gpsimd.dma_start(glo_sb[:, im], gl3[:, im]))
            sw_dmas.append(nc.gpsimd.dma_start(uhi_sb[:, im], u3[:, im]))
            sw_dmas.append(nc.gpsimd.dma_start(ulo_sb[:, im], ul3[:, im]))

        # x pieces as separate write-once tensors (hazard tracking is
        # tensor-granular, so each matmul deps only its own piece's DMA);
        # out is a small ring that never aliases x, so out writes/stores
        # carry no x hazards at all
        xp = [
            wpool.tile([P, KH // 4, C], f8, name=f"xp{i}") for i in range(8)
        ]
        out_r = wpool.tile([P, KH // 4, C], bf16, name="out_r")

        # stream order is what the (serialized) DMA engine model follows:
        # first gate weights + the xh pieces that unlock P1/P3 work, then
        # up weights + the xl pieces, then the weight tail. x pieces ride
        # the otherwise idle ACT/DVE HWDGE rings.
        R = KH // 4  # x piece rows
        x_dmas = []

        def xblk(i):
            src = x3h if i < 4 else x3l
            j = i % 4
            x_dmas.append(
                nc.gpsimd.dma_start(xp[i][:], src[:, j * R : (j + 1) * R]))

        # tightest prologue: the first chain's operands arrive in exactly
        # the order its matmuls consume them; the very first two ride the
        # SP HWDGE ring, which starts ~1.4us sooner than the SWDGE ring
        x_dmas.append(nc.sync.dma_start(xp[0][:], x3h[:, 0:R]))
        sw_dmas.append(nc.sync.dma_start(ghi_sb[:, 0], g3[:, 0]))
        sw_dmas.append(nc.gpsimd.dma_start(glo_sb[:, 0], gl3[:, 0]))
        xblk(1)
        sw_dmas.append(nc.gpsimd.dma_start(uhi_sb[:, 0], u3[:, 0]))
        xblk(2)
        sw_dmas.append(nc.gpsimd.dma_start(ulo_sb[:, 0], ul3[:, 0]))
        xblk(3)
        # interleave the second weight block with the xl pieces: each
        # stream stays just ahead of the matmuls that consume it
        sw_dmas.append(nc.gpsimd.dma_start(ghi_sb[:, 1], g3[:, 1]))
        xblk(4)
        sw_dmas.append(nc.gpsimd.dma_start(glo_sb[:, 1], gl3[:, 1]))
        xblk(5)
        xblk(6)
        sw_dmas.append(nc.gpsimd.dma_start(uhi_sb[:, 1], u3[:, 1]))
        xblk(7)
        sw_dmas.append(nc.gpsimd.dma_start(ulo_sb[:, 1], ul3[:, 1]))
        for im in range(2, KI):
            wblk(im)
        sw_dmas.append(nc.gpsimd.dma_start(
            dhi_sb[:], dhi_d[:].rearrange("p (i h) -> p i h", i=KI2)))
        sw_dmas.append(nc.gpsimd.dma_start(
            dlo_sb[:], dlo_d[:].rearrange("p (i h) -> p i h", i=KI2)))

        def xsl(plane, k, o, w):
            # x operand slice for k-subtile pair (k, k+1) of a plane
            t = xp[plane * 4 + k // R]
            kk = k % R
            return t[:, kk : kk + 2, o : o + w]

        ivl_sb = ipool.tile([P, KI, 2, C], f8, name="ivl_sb")
        # double-depth rotating staging (im%2, chunk): a slot's DVE readers
        # from round im-2 are absorbed into the PE proc by a per-im fence
        # that never stalls (two full rounds of PE work in between), so the
        # ACT rewrites and DVE ops carry no cross-WAR waits of their own
        t1_sb = spool.tile([P, 2, NJ, 512], bf16, name="t1_sb")
        v_sb = spool.tile([P, 2, NJ, 512], bf16, name="v_sb")
        # fence scratch: never written by real producers, so fence
        # reads/writes of their disjoint per-round columns carry no deps
        # beyond the explicit edges. Wait dedup is exact-instruction per
        # engine, so each fence lists the precise producer set (one
        # semaphore) it absorbs for its queue.
        fence_t = spool.tile([P, 64], bf16, name="fence_t")
        if _INIT_FENCE:
            nc.vector.memset(fence_t[:], 0.0)
        last_tt = [None]
        dve_by_im = {}
        act_by_im = {}

        # PE warm-up: the Tensor engine runs at half clock for its first
        # 3us of continuous activity. Burn the ramp on dependency-free
        # dummy matmuls (garbage operands into a psum bank the real work
        # later resets with start=True) while the first DMAs are in
        # flight, so every real matmul runs at full speed.
        warm_sb = spool.tile([P, 2, 512], f8, name="warm_sb")
        nc.vector.memset(warm_sb[:], 0.0)
        pwarm = ppool2.tile([P, 512], f32, tag="c0", name="pwarm")
        for _ in range(22):
            nc.tensor.matmul(
                pwarm[:], warm_sb[:, :, 0:P], warm_sb[:],
                start=True, stop=True, perf_mode=DR)

        # ---- phase 1: interT = silu(gateT) * upT, requantized hi/lo ----
        def chain_p13(psum, whi, wlo, im, o, w):
            # hi-plane products grouped by x piece (so each arriving piece
            # unlocks work during the prologue); opens the psum group
            first = True
            for k in range(0, KH, 2):
                nc.tensor.matmul(
                    psum[:, :w], whi[:, im, k : k + 2], xsl(0, k, o, w),
                    start=first, stop=False, perf_mode=DR)
                first = False
                nc.tensor.matmul(
                    psum[:, :w], wlo[:, im, k : k + 2], xsl(0, k, o, w),
                    start=False, stop=False, perf_mode=DR)

        def chain_p2(psum, whi, im, o, w):
            # lo-x-plane products; closes the psum group. Emitted after
            # ALL chains' p13 stages so the in-order PE never stalls on
            # the xl pieces while xh-only work remains.
            for k in range(0, KH, 2):
                nc.tensor.matmul(
                    psum[:, :w], whi[:, im, k : k + 2], xsl(1, k, o, w),
                    start=False, stop=(k == KH - 2), perf_mode=DR)

        for im in range(KI):
            s = im % 2
            if im >= 2:
                # absorb round im-2's exact producer sets into each queue:
                # af takes its DVE ops (one DVE sem), af2 its ACT ops (own
                # sem), df its DVE ops for the DVE queue. The staging
                # slots' WAR/WAW hazards this round then dedup away and
                # every real ACT/DVE instruction keeps a single wait.
                ca = 16 + 2 * (im - 2)
                af = nc.scalar.copy(
                    fence_t[:1, ca : ca + 2], fence_t[:1, ca : ca + 2])
                for bi in dve_by_im[im - 2]:
                    add_dep_helper(af.ins, bi.ins, sync=True,
                                   reason="act-dve slot fence")
                ca += 20
                af2 = nc.scalar.copy(
                    fence_t[:1, ca : ca + 2], fence_t[:1, ca : ca + 2])
                for bi in act_by_im[im - 2]:
                    add_dep_helper(af2.ins, bi.ins, sync=True,
                                   reason="act-act slot fence")
                # df's WAW against the previous df lands on the same DVE
                # semaphore as its edges, so one fixed column suffices
                df = nc.vector.tensor_copy(
                    fence_t[:1, 56:58], fence_t[:1, 56:58])
                for bi in dve_by_im[im - 2]:
                    add_dep_helper(df.ins, bi.ins, sync=True,
                                   reason="dve slot fence")
            pg = [
                ppool.tile([P, w], f32, tag=f"a{j}", name=f"pg{j}")
                for j, (_, w) in enumerate(chunks)
            ]
            pu = [
                ppool.tile([P, w], f32, tag=f"b{j}", name=f"pu{j}")
                for j, (_, w) in enumerate(chunks)
            ]
            for j, (o, w) in enumerate(chunks):
                chain_p13(pg[j], ghi_sb, glo_sb, im, o, w)
                chain_p13(pu[j], uhi_sb, ulo_sb, im, o, w)
            for j, (o, w) in enumerate(chunks):
                chain_p2(pg[j], ghi_sb, im, o, w)
                chain_p2(pu[j], uhi_sb, im, o, w)
            dve_by_im[im] = []
            act_by_im[im] = []
            for j, (o, w) in enumerate(chunks):
                # t1 = silu(g) (scale folds the fp8 scaling out before the
                # nonlinearity); v = u * S_I, then in-place v = inter * S_I
                a1 = nc.scalar.activation(
                    t1_sb[:, s, j, :w], pg[j][:],
                    mybir.ActivationFunctionType.Silu, scale=1.0 / (S_W * S_X))
                a2 = nc.scalar.activation(
                    v_sb[:, s, j, :w], pu[j][:],
                    mybir.ActivationFunctionType.Copy, scale=S_I / (S_W * S_X))
                # DVE chain: mul reads two ACT products (one collapsed sem),
                # copy/sub read only DVE-produced data (self sem)
                d1 = nc.vector.tensor_mul(
                    v_sb[:, s, j, :w], t1_sb[:, s, j, :w], v_sb[:, s, j, :w])
                d2 = nc.vector.tensor_copy(
                    ivl_sb[:, im, 0, o : o + w], v_sb[:, s, j, :w])
                d3 = nc.vector.tensor_sub(
                    ivl_sb[:, im, 1, o : o + w],
                    v_sb[:, s, j, :w],
                    ivl_sb[:, im, 0, o : o + w],
                )
                last_tt[0] = d3
                act_by_im[im] += [a1, a2]
                dve_by_im[im] += [d1, d2, d3]

        # ---- phase 2: outT = down_w @ interT ----
        # absorb every x-DMA completion into the ACT proc and the SP queue
        # (exact-instruction dedup: later out writes/stores on the aliased
        # slab then drop their WAW-vs-x deps and carry only PE/ACT). The
        # ACT fences use dep-free fence columns; the SP dummies read tiny
        # x slices whose RAW lands on the same DMA they absorb.

        # absorb the last TT's DVE tick into the PE proc so phase-2 matmuls
        # wait only on their dw DMA lane
        pe_fence = nc.tensor.ldweights(ghi_sb[:, 0, 0, 0:1])
        add_dep_helper(pe_fence.ins, last_tt[0].ins, sync=True, reason="pe fence")
        hw_dmas = list(x_dmas)
        tail_insts = []
        last_mm = []
        for hm in range(KH):
            po = [
                ppool2.tile([P, w], f32, tag=f"c{j}", name=f"po{j}")
                for j, (_, w) in enumerate(chunks)
            ]
            for j, (o, w) in enumerate(chunks):
                hs = hm * P
                n = 0
                for k in range(0, KI - 1, 2):
                    nc.tensor.matmul(
                        po[j][:, :w], dhi_sb[:, k : k + 2, hs : hs + P],
                        ivl_sb[:, k : k + 2, 0, o : o + w],
                        start=(n == 0), stop=False, perf_mode=DR)
                    n += 1
                for k in range(0, KI - 1, 2):
                    nc.tensor.matmul(
                        po[j][:, :w], dhi_sb[:, k : k + 2, hs : hs + P],
                        ivl_sb[:, k : k + 2, 1, o : o + w],
                        start=False, stop=False, perf_mode=DR)
                for k in range(0, KI - 1, 2):
                    nc.tensor.matmul(
                        po[j][:, :w], dlo_sb[:, k : k + 2, hs : hs + P],
                        ivl_sb[:, k : k + 2, 0, o : o + w],
                        start=False, stop=False, perf_mode=DR)
                # odd last subtile: its hi/lo planes are adjacent in ivl,
                # paired against the duplicated stationary slot KI..KI+1
                nc.tensor.matmul(
                    po[j][:, :w], dhi_sb[:, KI - 1 : KI + 1, hs : hs + P],
                    ivl_sb[:, KI - 1, 0:2, o : o + w],
                    start=False, stop=False, perf_mode=DR)
                last_mm.append(nc.tensor.matmul(
                    po[j][:, :w], dlo_sb[:, KI - 1 : KI + 1, hs : hs + P],
                    ivl_sb[:, KI - 1, 0:2, o : o + w],
                    start=False, stop=True, perf_mode=DR))
                del last_mm[:-1]
            q, qr = hm // (KH // 4), hm % (KH // 4)
            if qr == 0 and q >= 1:
                # ring handoff: absorb the previous quarter's copies (ACT
                # self sem) and its two pair stores (one fence per DMA
                # semaphore) so this quarter's copies keep their single
                # PE wait
                f3a = nc.scalar.copy(fence_t[:1, 0:2], fence_t[:1, 0:2])
                for bi in tail_insts[-2 * (KH // 4):]:
                    add_dep_helper(f3a.ins, bi.ins, sync=True,
                                   reason="ring copy fence")
                ca = 2 + 2 * (q - 1)
                f3b = nc.scalar.copy(
                    fence_t[:1, ca : ca + 2], fence_t[:1, ca : ca + 2])
                add_dep_helper(f3b.ins, hw_dmas[-1].ins, sync=True,
                               reason="ring store fence")
            for j, (o, w) in enumerate(chunks):
                tail_insts.append(nc.scalar.activation(
                    out_r[:, qr, o : o + w], po[j][:],
                    mybir.ActivationFunctionType.Copy, scale=1.0 / (S_W * S_I)))
            # quarter stores for q0-q2; the last quarter tapers (pair,
            # single, single) so the tail store after the final copy is
            # one row. 6 SP DMAs total, each on a fresh HW lane.
            if q < 3:
                if qr == KH // 4 - 1:
                    hw_dmas.append(nc.sync.dma_start(
                        o3[:, hm - qr : hm + 1, :], out_r[:]
                    ))
            elif qr == 1:
                hw_dmas.append(nc.sync.dma_start(
                    o3[:, hm - 1 : hm + 1, :], out_r[:, 0:2]
                ))
            elif qr >= 2:
                hw_dmas.append(nc.sync.dma_start(
                    o3[:, hm : hm + 1, :], out_r[:, qr : qr + 1]
                ))

        # pre-drain: absorb every proc's final tick into the SP sequencer
        # one sync edge at a time, so the kernel-tail drain's waits (which
        # would exceed the instruction's wait slots) are all elided
        for insts in (sw_dmas, hw_dmas, [last_tt[0]], tail_insts[-2:], last_mm[-1:]):
            for bi in insts:
                if bi is None:
                    continue
                nop = nc.sync.nop()
                add_dep_helper(nop.ins, bi.ins, sync=True, reason="pre-drain")

    return nc


def _split_fp8(a, scale):
    """Split scale*a into hi/lo float8_e4m3 planes (as fp8 arrays)."""
    import ml_dtypes

    f8 = ml_dtypes.float8_e4m3
    v = (a * scale).astype(np.float32)
    hi = v.astype(f8)
    lo = (v - hi.astype(np.float32)).astype(f8)
    return hi, lo


def kernel(hidden_states, top_k_index, top_k_weights, gate_w, up_w, down_w):
    import ml_dtypes
    from concourse.bass_utils import run_bass_kernel_spmd

    hs = np.ascontiguousarray(np.asarray(hidden_states, dtype=np.float32))
    tki = np.asarray(top_k_index)
    tkw = np.asarray(top_k_weights, dtype=np.float32)
    gw = np.asarray(gate_w, dtype=np.float32)
    uw = np.asarray(up_w, dtype=np.float32)
    dw = np.asarray(down_w, dtype=np.float32)

    T, H = hs.shape
    E, I, _ = gw.shape
    P = 128
    KH = H // P
    KI = I // P
    KI2 = KI + 1

    tok_lists, w_lists = [], []
    for e in range(E):
        mask = tki == e
        toks = np.nonzero(mask.any(axis=1))[0]
        w = (tkw * mask).sum(axis=1)[toks].astype(np.float32)
        tok_lists.append(toks)
        w_lists.append(w)

    # capacity: smallest multiple of 16 covering the busiest expert
    # (1008 for the balanced T=4096, K=2, E=8 regime); experts with more
    # tokens spill into additional SPMD rounds
    maxn = max(len(t) for t in tok_lists)
    C = min(1024, -(-maxn // 16) * 16)
    n_rounds = max(1, -(-maxn // C))

    def pack_gu(a):  # [I, H] fp8 -> [P, KI*KH*P] im-block-major
        # column block im of a.T[H, I] as [P, KH, 128], contiguous per block
        b = a.T.reshape(KH, P, KI, P).transpose(1, 2, 0, 3)  # p, im, kh, q
        return np.ascontiguousarray(b.reshape(P, KI * KH * P))

    def pack_d(a):  # [H, I] fp8 -> [P, KI2*H] with duplicated last subtile
        b = a.T.reshape(KI, P, H).transpose(1, 0, 2)  # p, ki, h
        b = np.concatenate([b, b[:, KI - 1 : KI]], axis=1)  # dup slot
        return np.ascontiguousarray(b.reshape(P, KI2 * H))

    wmaps = []
    for e in range(E):
        ghi, glo = _split_fp8(gw[e], S_W)
        uhi, ulo = _split_fp8(uw[e], S_W)
        dhi, dlo = _split_fp8(dw[e], S_W)
        wmaps.append({
            "ghi": pack_gu(ghi), "glo": pack_gu(glo),
            "uhi": pack_gu(uhi), "ulo": pack_gu(ulo),
            "dhi": pack_d(dhi), "dlo": pack_d(dlo),
        })

    nc = _build_bass(C, H, I)
    out = np.zeros((T, H), np.float32)
    global _last_results
    for r in range(n_rounds):
        in_maps = []
        for e in range(E):
            toks = tok_lists[e][r * C : (r + 1) * C]
            xT = np.zeros((H, C), np.float32)
            xT[:, : len(toks)] = hs[toks].T
            xhi, xlo = _split_fp8(xT, S_X)

            def pack_x(a):  # [H, C] -> [P, KH*C]
                b = a.reshape(KH, P, C).transpose(1, 0, 2)
                return np.ascontiguousarray(b.reshape(P, KH * C))

            in_maps.append({"xhi": pack_x(xhi), "xlo": pack_x(xlo), **wmaps[e]})
        res = run_bass_kernel_spmd(nc, in_maps, core_ids=list(range(E)))
        _last_results = res
        for e in range(E):
            toks = tok_lists[e][r * C : (r + 1) * C]
            n = len(toks)
            if n == 0:
                continue
            o = np.asarray(res.results[e]["out"]).astype(np.float32)  # [P, KH*C]
            outT = o.reshape(P, KH, C).transpose(1, 0, 2).reshape(H, C)
            out[toks] += w_lists[e][r * C : r * C + n, None] * outT[:, :n].T
    return out
